# revision 14
# baseline (speedup 1.0000x reference)
"""Graphormer layer on 8 TRN2 NeuronCores.

Sharding: core c handles batch b = c//2 and query-row half qh = c%2 (1024 q
rows). All compute is in transposed (feature-on-partition) layout; the host
pre-transposes x and the influence slices and transposes per-core outputs
back during the gather. Host also rolls the node axis per core so each core's
own q rows sit at columns [0, 1024) — the device program is identical across
cores (attention over all keys is permutation-invariant; the influence k axis
is rolled identically).

v3 design notes:
  - The influence add is handled entirely multiplicatively:
    exp(s + LG) = exp(s)*EG with EG = exp(iw1*u + ib1). The host ships
    gaT = EG and gbT = EG*(iw2*u + ib2) in bf16 (same DMA bytes as the raw
    fp32 influence), so the device does NO influence prep at all — the PE
    identity-preload path and the GpSimd prep of earlier versions are gone.
  - The attention inner loop is software-pipelined: the QK matmuls of
    iteration i+1 are issued on the PE queue before the Z/WV matmuls of
    iteration i, so the PE never waits on the ACT exp / DVE multiply chain.
  - A subset of the Z-source multiplies runs on the otherwise-idle GpSimd
    engine to relieve the DVE.
  - LayerNorm is rebalanced across ACT (Square/Sqrt/scale) and DVE; bias
    adds and ReLU in the attention tail and FFN run on ACT (Copy/Identity/
    Relu/Square live in every ACT table, so only Exp<->Sqrt transitions
    reload tables: ~3 reloads per kernel).

Per core:
  ln1T = LayerNorm_T(xT)              (partition-dim LN via ones-matmul sums)
  QT/KT/V projections (bf16)
  per (qc, kc, head-pair):
    psum = KT_h.T @ QT_h              (bf16, contract d=32, row-packed)
    e = exp(psum)                     (ACT -> SBUF bf16)
    zsrc = e * EG[kc]                 (DVE/GpSimd bf16)
    f = e * (EG*G2)[kc]               (DVE bf16)
    Z  += ones.T @ zsrc               (bf16, col-packed per head)
    wv += V_kc-slice.T @ f            (bf16, col-packed per head)
  attn = (wv / Z) @ Wo + bo ; h = attn + xT_own
  out = W2.T-proj(relu(W1.T-proj(LN_T(h)) + b1)) + b2 + h
"""

import math

import numpy as np

import concourse.bass as bass
import concourse.bacc as bacc
import concourse.mybir as mybir
import concourse.tile as tile
from concourse.bass_utils import run_bass_kernel_spmd

B, N, E, H, D = 4, 2048, 256, 8, 32
NQ = N // 2          # q rows per core
QC = 512             # q window
NKC = N // 128       # 16 k-chunks
EC = 2               # feature chunks

f32 = mybir.dt.float32
bf16 = mybir.dt.bfloat16
FT = mybir.ActivationFunctionType
ALU = mybir.AluOpType

# vecs_sb column index: vec v, chunk c -> 2*v + c
V_G1, V_BETA1, V_G2, V_BETA2, V_BO, V_B1, V_B2 = range(7)


def is_preload(kc):
    """kc chunks whose influence-add goes through a PSUM identity preload
    (LG/G2 shipped) instead of the multiplicative EG path."""
    return False


def layer_norm_T(nc, pp, ps, x_chunks, win, wn, g_col, beta_col, vecs, ones,
                 eps_ap, out_chunks, act_heavy=True):
    """LayerNorm over the partition dim (E = 2 chunks) in T layout.

    x_chunks: 2 SBUF APs; normalizes cols [win:win+wn] -> out_chunks.
    Square/scale on ACT, rstd = 1/sqrt via ACT Sqrt + DVE reciprocal.
    """
    sq = ps.tile([128, 2 * wn], f32, name="lnsq", tag="lnsq")
    p_s = pp.tile([128, wn], f32, name="lnps", tag="lnps")
    p_sq = pp.tile([128, wn], f32, name="lnpsq", tag="lnpsq")
    for c in range(EC):
        xs = x_chunks[c][:, win:win + wn]
        if act_heavy:
            nc.scalar.activation(sq[:, c * wn:(c + 1) * wn], xs, FT.Square)
        else:
            nc.vector.tensor_mul(sq[:, c * wn:(c + 1) * wn], xs, xs)
        nc.tensor.matmul(p_s[:, :], ones[:, :], xs,
                         start=(c == 0), stop=(c == EC - 1))
    for c in range(EC):
        nc.tensor.matmul(p_sq[:, :], ones[:, :],
                         sq[:, c * wn:(c + 1) * wn],
                         start=(c == 0), stop=(c == EC - 1))
    mu = ps.tile([128, wn], f32, name="lnmu", tag="lnmu")
    mu2 = ps.tile([128, wn], f32, name="lnmu2", tag="lnmu2")
    if act_heavy:
        nc.scalar.activation(mu[:, :], p_s[:, :], FT.Copy, scale=1.0 / E)
        nc.scalar.activation(mu2[:, :], mu[:, :], FT.Square)
    else:
        nc.vector.tensor_scalar_mul(mu[:, :], p_s[:, :], 1.0 / E)
        nc.vector.tensor_mul(mu2[:, :], mu[:, :], mu[:, :])
    var = ps.tile([128, wn], f32, name="lnvar", tag="lnvar")
    nc.vector.scalar_tensor_tensor(var[:, :], p_sq[:, :], 1.0 / E, mu2[:, :],
                                   ALU.mult, ALU.subtract)
    sd = ps.tile([128, wn], f32, name="lnsd", tag="lnsd")
    nc.scalar.activation(sd[:, :], var[:, :], FT.Sqrt, bias=eps_ap)
    rstd = ps.tile([128, wn], f32, name="lnrstd", tag="lnrstd")
    nc.vector.reciprocal_approx_fast(rstd[:, :], sd[:, :])
    for c in range(EC):
        xs = x_chunks[c][:, win:win + wn]
        xm = ps.tile([128, wn], f32, name="lnxm", tag="lnxm")
        nc.vector.tensor_sub(xm[:, :], xs, mu[:, :])
        xm2 = ps.tile([128, wn], f32, name="lnxm2", tag="lnxm2")
        nc.vector.tensor_mul(xm2[:, :], xm[:, :], rstd[:, :])
        nc.vector.tensor_scalar(
            out_chunks[c][:, win:win + wn], xm2[:, :],
            vecs[:, 2 * g_col + c:2 * g_col + c + 1],
            vecs[:, 2 * beta_col + c:2 * beta_col + c + 1],
            ALU.mult, ALU.add)


def build_body(nc, tc, xT_d, gaT_d, gbT_d, w_d, vecs_d, ident_d, outT_d):
    persist_pools = []

    def ppool(name):
        p = tc.tile_pool(name=name, bufs=1)
        persist_pools.append(p)
        return p.__enter__()

    persist = ppool("persist")

    # ---- persistent SBUF ----
    qt = [persist.tile([128, NQ], bf16, name=f"qt{c}", tag=f"qt{c}")
          for c in range(EC)]
    kt = [persist.tile([128, N], bf16, name=f"kt{c}", tag=f"kt{c}")
          for c in range(EC)]
    xt = [persist.tile([128, N], f32, name=f"xt{c}", tag=f"xt{c}")
          for c in range(EC)]
    v_sb = [persist.tile([128, E], bf16, name=f"v{k}", tag=f"v{k}") for k in range(NKC)]
    ga_sb = [persist.tile([128, NQ], bf16, name=f"ga_{k}", tag=f"ga_{k}") for k in range(NKC)]
    gb_sb = [persist.tile([128, NQ], bf16, name=f"gb_{k}", tag=f"gb_{k}") for k in range(NKC)]
    w_sb = {n: persist.tile([128, 2 * E], f32, name=f"w_{n}", tag=f"w_{n}") for n in w_d}
    w_bf = {n: persist.tile([128, 2 * E], bf16, name=f"wbf_{n}", tag=f"wbf_{n}")
            for n in w_d}
    vecs = persist.tile([128, 14], f32, name="vecs", tag="vecs")
    id_bf = persist.tile([128, 128], bf16, name="id_bf", tag="id_bf")
    ones = persist.tile([128, 128], f32, name="ones", tag="ones")
    ones_bf = persist.tile([128, 32], bf16, name="ones_bf", tag="ones_bf")
    h_sb = [[persist.tile([128, QC], f32, name=f"h{q}{c}", tag=f"h{q}{c}") for c in range(EC)]
            for q in range(2)]

    # ---- small loads ----
    for n in w_d:
        for c in range(EC):
            nc.sync.dma_start(w_sb[n][:, E * c:E * (c + 1)],
                              w_d[n][128 * c:128 * (c + 1), :])
    nc.sync.dma_start(vecs[:, :], vecs_d[:, :])
    idt = persist.tile([128, 128], f32, name="id_f32", tag="id_f32")
    nc.sync.dma_start(idt[:, :], ident_d[:, :])
    nc.vector.tensor_copy(id_bf[:, :], idt[:, :])
    nc.vector.memset(ones[:, :], 1.0)
    nc.vector.memset(ones_bf[:, :], 1.0)
    eps_t = persist.tile([128, 1], f32, name="eps_t", tag="eps_t")
    nc.vector.memset(eps_t[:, :], 1e-5)
    for n in w_d:
        nc.vector.tensor_copy(w_bf[n][:, :], w_sb[n][:, :])

    def emit_prep(kc):
        """Influence gates for chunk kc: just two bf16 DMA loads."""
        nc.sync.dma_start(ga_sb[kc][:, :], gaT_d[128 * kc:128 * (kc + 1), :])
        nc.sync.dma_start(gb_sb[kc][:, :], gbT_d[128 * kc:128 * (kc + 1), :])

    # ---- stage B/C: LN1 + projections ----
    with tc.tile_pool(name="ln_pool", bufs=1) as lnp, \
         tc.tile_pool(name="ln_psum", bufs=2, space="PSUM") as ln_pp, \
         tc.tile_pool(name="ln_sbuf", bufs=2) as ln_ps, \
         tc.tile_pool(name="proj_psum", bufs=2, space="PSUM") as proj_psum:
        ln1 = [lnp.tile([128, N], bf16, name=f"ln1{c}", tag=f"ln1{c}") for c in range(EC)]
        for c in range(EC):
            nc.sync.dma_start(xt[c][:, :], xT_d[128 * c:128 * (c + 1), :])
        # gate loads queue behind xt/weights; they land well before use
        for kc in range(NKC):
            emit_prep(kc)
        for w in range(N // 512):
            layer_norm_T(nc, ln_pp, ln_ps, xt, 512 * w, 512, V_G1, V_BETA1,
                         vecs, ones, eps_t[:, :], ln1, act_heavy=False)
        for fc in range(EC):
            for qw in range(NQ // 512):
                pq = proj_psum.tile([128, 512], f32, name="proj", tag="proj")
                for ec in range(EC):
                    nc.tensor.matmul(
                        pq[:, :],
                        w_bf["Wq"][:, E * ec + 128 * fc:E * ec + 128 * (fc + 1)],
                        ln1[ec][:, 512 * qw:512 * (qw + 1)],
                        start=(ec == 0), stop=(ec == EC - 1))
                nc.vector.tensor_copy(qt[fc][:, 512 * qw:512 * (qw + 1)], pq[:, :])
        for fc in range(EC):
            for kw in range(N // 512):
                pk = proj_psum.tile([128, 512], f32, name="proj", tag="proj")
                for ec in range(EC):
                    nc.tensor.matmul(
                        pk[:, :],
                        w_bf["Wk"][:, E * ec + 128 * fc:E * ec + 128 * (fc + 1)],
                        ln1[ec][:, 512 * kw:512 * (kw + 1)],
                        start=(ec == 0), stop=(ec == EC - 1))
                nc.vector.tensor_copy(kt[fc][:, 512 * kw:512 * (kw + 1)], pk[:, :])
        for k in range(NKC):
            pv = proj_psum.tile([128, E], f32, name="projv", tag="projv")
            for ec in range(EC):
                nc.tensor.matmul(
                    pv[:, :],
                    ln1[ec][:, 128 * k:128 * (k + 1)],
                    w_bf["Wv"][:, E * ec:E * (ec + 1)],
                    start=(ec == 0), stop=(ec == EC - 1))
            nc.vector.tensor_copy(v_sb[k][:, :], pv[:, :])

    # ---- stage E: attention (software pipelined) + F: FFN ----
    _cms = []

    def mpool(name, bufs, space="SBUF"):
        cm = tc.tile_pool(name=name, bufs=bufs, space=space)
        _cms.append(cm)
        return cm.__enter__()

    sp = mpool("score_psum", 2, "PSUM")
    ap_ = mpool("acc_psum", 1, "PSUM")
    efp = mpool("ef_sbuf", 8)
    asb = mpool("att_sbuf", 2)

    acc = {}  # qc -> (wv_ps, z_ps)

    def emit_qk(qc, kc, half):
        """PE part 1 of iteration (qc, kc, half): 4 QK matmuls for heads
        4*half .. 4*half+3 (row-packed, start=True)."""
        q0 = QC * qc
        pre = is_preload(kc)
        sts = []
        for s in range(2):
            st = sp.tile([128, 2 * QC], f32, name="score", tag="score")
            sts.append(st)
        if pre:
            for st in sts:
                for j in range(2):
                    nc.tensor.matmul(
                        st[:, QC * j:QC * (j + 1)],
                        id_bf[:, :],
                        ga_sb[kc][:, q0:q0 + QC],
                        start=True, stop=False)
        for s in range(2):
            st = sts[s]
            for j in range(2):
                h = 4 * half + 2 * s + j
                c, hh = h // 4, 32 * (h % 4)
                nc.tensor.matmul(
                    st[:, QC * j:QC * (j + 1)],
                    kt[c][hh:hh + 32, 128 * kc:128 * (kc + 1)],
                    qt[c][hh:hh + 32, q0:q0 + QC],
                    start=not pre, stop=True,
                    skip_group_check=True, tile_position=(hh, 0))
        return (qc, kc, half, sts)

    def emit_rest(state):
        """ACT exp, DVE/GpSimd multiplies, PE Z/WV accumulation."""
        qc, kc, half, sts = state
        q0 = QC * qc
        wv_ps, z_ps = acc[qc]
        gab = ga_sb[kc][:, q0:q0 + QC].rearrange(
            "p (o q) -> p o q", o=1).broadcast_to([128, 2, QC])
        gbb = gb_sb[kc][:, q0:q0 + QC].rearrange(
            "p (o q) -> p o q", o=1).broadcast_to([128, 2, QC])
        for s in range(2):
            st = sts[s]
            e = efp.tile([128, 2 * QC], bf16, name="e", tag="e")
            nc.scalar.activation(e[:, :], st[:, :], FT.Exp)
            er = e[:, :].rearrange("p (o q) -> p o q", o=2)
            if is_preload(kc):
                zsrc = e   # e already includes the influence add
            else:
                zsrc = efp.tile([128, 2 * QC], bf16, name="t", tag="t")
                zeng = nc.gpsimd if (s == 1 and kc % 4 != 0) else nc.vector
                zeng.tensor_tensor(
                    zsrc[:, :].rearrange("p (o q) -> p o q", o=2),
                    er, gab, ALU.mult)
            for j in range(2):
                h = 4 * half + 2 * s + j
                s_, hh = h // 4, 32 * (h % 4)
                nc.tensor.matmul(
                    z_ps[s_][hh:hh + 32, :],
                    ones_bf[:, :],
                    zsrc[:, QC * j:QC * (j + 1)],
                    start=(kc == 0), stop=(kc == NKC - 1),
                    skip_group_check=True, tile_position=(0, hh))
            f = efp.tile([128, 2 * QC], bf16, name="f", tag="f")
            nc.vector.tensor_tensor(
                f[:, :].rearrange("p (o q) -> p o q", o=2),
                er, gbb, ALU.mult)
            for j in range(2):
                h = 4 * half + 2 * s + j
                s_, hh = h // 4, 32 * (h % 4)
                nc.tensor.matmul(
                    wv_ps[s_][hh:hh + 32, :],
                    v_sb[kc][:, 32 * h:32 * h + 32],
                    f[:, QC * j:QC * (j + 1)],
                    start=(kc == 0), stop=(kc == NKC - 1),
                    skip_group_check=True, tile_position=(0, hh))

    def emit_attn_tail(qc):
        """normalize + Wo projection + bias + residual -> h_sb[qc]."""
        q0 = QC * qc
        wv_ps, z_ps = acc[qc]
        on = []
        for s in range(2):
            zr = asb.tile([128, QC], f32, name=f"zr{s}", tag=f"zr{s}")
            nc.vector.reciprocal_approx_fast(zr[:, :], z_ps[s][:, :])
            o = asb.tile([128, QC], bf16, name=f"on{s}", tag=f"on{s}")
            nc.vector.tensor_mul(o[:, :], wv_ps[s][:, :], zr[:, :])
            on.append(o)
        for fc in range(EC):
            po = sp.tile([128, QC], f32, name="score", tag="score")
            for ec in range(EC):
                nc.tensor.matmul(
                    po[:, :],
                    w_bf["Wo"][:, E * ec + 128 * fc:E * ec + 128 * (fc + 1)],
                    on[ec][:, :],
                    start=(ec == 0), stop=(ec == EC - 1))
            ta = asb.tile([128, QC], f32, name="tattn", tag="tattn")
            nc.scalar.activation(ta[:, :], po[:, :], FT.Identity,
                                 bias=vecs[:, 2 * V_BO + fc:2 * V_BO + fc + 1])
            nc.vector.tensor_add(h_sb[qc][fc][:, :], ta[:, :],
                                 xt[fc][:, q0:q0 + QC])

    def emit_ffn(qc, ln_pp2, ln_ps2, fp_, fs):
        """LN2 + FFN + residual + store for one q half."""
        ln2 = [fs.tile([128, QC], bf16, name=f"ln2{c}", tag=f"ln2{c}") for c in range(EC)]
        layer_norm_T(nc, ln_pp2, ln_ps2, h_sb[qc], 0, QC, V_G2, V_BETA2,
                     vecs, ones, eps_t[:, :], ln2)
        z1 = [fs.tile([128, QC], bf16, name=f"z1{c}", tag=f"z1{c}") for c in range(EC)]
        for fc in range(EC):
            p1 = fp_.tile([128, QC], f32, name="ffn", tag="ffn")
            for ec in range(EC):
                nc.tensor.matmul(
                    p1[:, :],
                    w_bf["W1"][:, E * ec + 128 * fc:E * ec + 128 * (fc + 1)],
                    ln2[ec][:, :],
                    start=(ec == 0), stop=(ec == EC - 1))
            nc.scalar.activation(z1[fc][:, :], p1[:, :], FT.Relu,
                                 bias=vecs[:, 2 * V_B1 + fc:2 * V_B1 + fc + 1])
        for fc in range(EC):
            p2 = fp_.tile([128, QC], f32, name="ffn", tag="ffn")
            for ec in range(EC):
                nc.tensor.matmul(
                    p2[:, :],
                    w_bf["W2"][:, E * ec + 128 * fc:E * ec + 128 * (fc + 1)],
                    z1[ec][:, :],
                    start=(ec == 0), stop=(ec == EC - 1))
            t2 = fs.tile([128, QC], f32, name="t2", tag="t2")
            nc.scalar.activation(t2[:, :], p2[:, :], FT.Identity,
                                 bias=vecs[:, 2 * V_B2 + fc:2 * V_B2 + fc + 1])
            of = fs.tile([128, QC], f32, name="of", tag="of")
            nc.gpsimd.tensor_add(of[:, :], t2[:, :], h_sb[qc][fc][:, :])
            nc.sync.dma_start(
                outT_d[128 * fc:128 * (fc + 1), QC * qc:QC * (qc + 1)],
                of[:, :])

    # ---- main software-pipelined loop ----
    for qc in range(2):
        wv_ps = [ap_.tile([128, QC], f32, name=f"wv{s}", tag=f"wv{s}") for s in range(2)]
        z_ps = [ap_.tile([128, QC], f32, name=f"z{s}", tag=f"z{s}") for s in range(2)]
        acc[qc] = (wv_ps, z_ps)
        pend = None
        for kc in range(NKC):
            for half in range(2):
                cur = emit_qk(qc, kc, half)
                if pend is not None:
                    emit_rest(pend)
                pend = cur
        emit_rest(pend)
        emit_attn_tail(qc)

    for cm in reversed(_cms):
        cm.__exit__(None, None, None)

    # ---- stage F: LN2 + FFN + residual + store (own PSUM scope) ----
    with tc.tile_pool(name="ln_psum2", bufs=2, space="PSUM") as ln_pp2, \
         tc.tile_pool(name="ln_sbuf2", bufs=2) as ln_ps2, \
         tc.tile_pool(name="ffn_psum", bufs=2, space="PSUM") as fp_, \
         tc.tile_pool(name="ffn_sbuf", bufs=2) as fs:
        for qc in range(2):
            emit_ffn(qc, ln_pp2, ln_ps2, fp_, fs)

    for p in reversed(persist_pools):
        p.__exit__(None, None, None)


def build_nc():
    nc = bacc.Bacc(
        "TRN2",
        target_bir_lowering=False,
        debug=False,
        enable_asserts=False,
        num_devices=8,
    )
    xT_d = nc.dram_tensor("xT", [E, N], f32, kind="ExternalInput").ap()
    gaT_d = nc.dram_tensor("gaT", [N, NQ], bf16, kind="ExternalInput").ap()
    gbT_d = nc.dram_tensor("gbT", [N, NQ], bf16, kind="ExternalInput").ap()
    w_d = {
        name: nc.dram_tensor(name, [E, E], f32, kind="ExternalInput").ap()
        for name in ("Wq", "Wk", "Wv", "Wo", "W1", "W2")
    }
    vecs_d = nc.dram_tensor("vecs", [128, 14], f32, kind="ExternalInput").ap()
    ident_d = nc.dram_tensor("ident", [128, 128], f32, kind="ExternalInput").ap()
    outT_d = nc.dram_tensor("outT", [E, NQ], f32, kind="ExternalOutput").ap()

    with tile.TileContext(nc) as tc:
        build_body(nc, tc, xT_d, gaT_d, gbT_d, w_d, vecs_d, ident_d, outT_d)
    nc.compile()
    return nc


def host_shard(inputs):
    """Build the 8 per-core input maps (see module docstring for the roll)."""
    x = np.asarray(inputs["x"], np.float32)
    infl = np.asarray(inputs["influence_matrix"], np.float32)
    iw1 = np.float32(inputs["iw1"])
    ib1 = np.float32(inputs["ib1"])
    iw2 = np.float32(inputs["iw2"])
    ib2 = np.float32(inputs["ib2"])
    vec_list = ["g1", "beta1", "g2", "beta2", "bo", "b1", "b2"]
    vecs_np = np.empty((128, 14), np.float32)
    for vi, nm in enumerate(vec_list):
        v = np.asarray(inputs[nm], np.float32).reshape(E)
        vecs_np[:, 2 * vi] = v[:128]
        vecs_np[:, 2 * vi + 1] = v[128:]
    ws = {n: np.ascontiguousarray(np.asarray(inputs[n], np.float32))
          for n in ("Wq", "Wv", "Wk", "Wo", "W1", "W2")}
    ws["Wq"] = ws["Wq"] / math.sqrt(D)

    # influence gates (shared across heads): EG = exp(iw1*u+ib1),
    # GB = EG*(iw2*u+ib2); shipped in bf16 per-core slices.
    import ml_dtypes
    lg = iw1 * infl + ib1
    g2 = iw2 * infl + ib2
    eg = np.exp(lg, dtype=np.float32)
    egg2 = eg * g2
    # per-k-chunk parity: preload chunks ship (LG, G2); others (EG, EG*G2).
    # The k-chunk index on the device is along the FIRST axis of the
    # transposed [N(k), NQ] slice, i.e. the original column axis of infl —
    # which is rolled per core. Build full-size gate tensors per core below.
    eg_bf = eg.astype(ml_dtypes.bfloat16)
    gb_bf = egg2.astype(ml_dtypes.bfloat16)
    lg_bf = lg.astype(ml_dtypes.bfloat16)
    g2_bf = g2.astype(ml_dtypes.bfloat16)

    in_maps = []
    for core in range(8):
        b, qh = core // 2, core % 2
        qoff = qh * NQ
        xb = np.roll(x[b], -qoff, axis=0)          # [N, E], own rows first
        xT = np.ascontiguousarray(xb.T)            # [E, N]
        gaT = np.ascontiguousarray(
            np.roll(eg_bf[b][qoff:qoff + NQ, :], -qoff, axis=1).T)  # [N(k), NQ]
        gbT = np.ascontiguousarray(
            np.roll(gb_bf[b][qoff:qoff + NQ, :], -qoff, axis=1).T)
        lgT = np.ascontiguousarray(
            np.roll(lg_bf[b][qoff:qoff + NQ, :], -qoff, axis=1).T)
        g2T = np.ascontiguousarray(
            np.roll(g2_bf[b][qoff:qoff + NQ, :], -qoff, axis=1).T)
        m = {"xT": xT, "gaT": gaT, "gbT": gbT, "vecs": vecs_np,
             "ident": np.eye(128, dtype=np.float32)}
        m.update(ws)
        in_maps.append(m)
    return in_maps


_NC_CACHE = []


def kernel(**inputs):
    if not _NC_CACHE:
        _NC_CACHE.append(build_nc())
    nc = _NC_CACHE[0]
    in_maps = host_shard(inputs)
    res = run_bass_kernel_spmd(nc, in_maps, core_ids=list(range(8)))
    out = np.empty((B, N, E), np.float32)
    for core in range(8):
        b, qh = core // 2, core % 2
        out[b, qh * NQ:(qh + 1) * NQ, :] = np.asarray(
            res.results[core]["outT"], np.float32).T
    return out


# revision 15
# speedup vs baseline: 1.0652x; 1.0652x over previous
"""Graphormer layer on 8 TRN2 NeuronCores.

Sharding: core c handles batch b = c//2 and query-row half qh = c%2 (1024 q
rows). All compute is in transposed (feature-on-partition) layout; the host
pre-transposes x and the influence slices and transposes per-core outputs
back during the gather. Host also rolls the node axis per core so each core's
own q rows sit at columns [0, 1024) — the device program is identical across
cores (attention over all keys is permutation-invariant; the influence k axis
is rolled identically).

v3 design notes:
  - The influence add is handled entirely multiplicatively:
    exp(s + LG) = exp(s)*EG with EG = exp(iw1*u + ib1). The host ships
    gaT = EG and gbT = EG*(iw2*u + ib2) in bf16 (same DMA bytes as the raw
    fp32 influence), so the device does NO influence prep at all — the PE
    identity-preload path and the GpSimd prep of earlier versions are gone.
  - The attention inner loop is software-pipelined: the QK matmuls of
    iteration i+1 are issued on the PE queue before the Z/WV matmuls of
    iteration i, so the PE never waits on the ACT exp / DVE multiply chain.
  - A subset of the Z-source multiplies runs on the otherwise-idle GpSimd
    engine to relieve the DVE.
  - LayerNorm is rebalanced across ACT (Square/Sqrt/scale) and DVE; bias
    adds and ReLU in the attention tail and FFN run on ACT (Copy/Identity/
    Relu/Square live in every ACT table, so only Exp<->Sqrt transitions
    reload tables: ~3 reloads per kernel).

Per core:
  ln1T = LayerNorm_T(xT)              (partition-dim LN via ones-matmul sums)
  QT/KT/V projections (bf16)
  per (qc, kc, head-pair):
    psum = KT_h.T @ QT_h              (bf16, contract d=32, row-packed)
    e = exp(psum)                     (ACT -> SBUF bf16)
    zsrc = e * EG[kc]                 (DVE/GpSimd bf16)
    f = e * (EG*G2)[kc]               (DVE bf16)
    Z  += ones.T @ zsrc               (bf16, col-packed per head)
    wv += V_kc-slice.T @ f            (bf16, col-packed per head)
  attn = (wv / Z) @ Wo + bo ; h = attn + xT_own
  out = W2.T-proj(relu(W1.T-proj(LN_T(h)) + b1)) + b2 + h
"""

import math

import numpy as np

import concourse.bass as bass
import concourse.bacc as bacc
import concourse.mybir as mybir
import concourse.tile as tile
from concourse.bass_utils import run_bass_kernel_spmd

B, N, E, H, D = 4, 2048, 256, 8, 32
NQ = N // 2          # q rows per core
QC = 512             # q window
NKC = N // 128       # 16 k-chunks
EC = 2               # feature chunks

f32 = mybir.dt.float32
bf16 = mybir.dt.bfloat16
FT = mybir.ActivationFunctionType
ALU = mybir.AluOpType

# vecs_sb column index: vec v, chunk c -> 2*v + c
V_G1, V_BETA1, V_G2, V_BETA2, V_BO, V_B1, V_B2 = range(7)


def is_preload(kc):
    """kc chunks whose influence-add goes through a PSUM identity preload
    (LG/G2 shipped) instead of the multiplicative EG path."""
    return False


def layer_norm_T(nc, pp, ps, x_chunks, win, wn, g_col, beta_col, vecs, ones,
                 eps_ap, out_chunks, act_heavy=True):
    """LayerNorm over the partition dim (E = 2 chunks) in T layout.

    x_chunks: 2 SBUF APs; normalizes cols [win:win+wn] -> out_chunks.
    Square/scale on ACT, rstd = 1/sqrt via ACT Sqrt + DVE reciprocal.
    """
    sq = ps.tile([128, 2 * wn], f32, name="lnsq", tag="lnsq")
    p_s = pp.tile([128, wn], f32, name="lnps", tag="lnps")
    p_sq = pp.tile([128, wn], f32, name="lnpsq", tag="lnpsq")
    for c in range(EC):
        xs = x_chunks[c][:, win:win + wn]
        if act_heavy:
            nc.scalar.activation(sq[:, c * wn:(c + 1) * wn], xs, FT.Square)
        else:
            nc.vector.tensor_mul(sq[:, c * wn:(c + 1) * wn], xs, xs)
        nc.tensor.matmul(p_s[:, :], ones[:, :], xs,
                         start=(c == 0), stop=(c == EC - 1))
    for c in range(EC):
        nc.tensor.matmul(p_sq[:, :], ones[:, :],
                         sq[:, c * wn:(c + 1) * wn],
                         start=(c == 0), stop=(c == EC - 1))
    mu = ps.tile([128, wn], f32, name="lnmu", tag="lnmu")
    mu2 = ps.tile([128, wn], f32, name="lnmu2", tag="lnmu2")
    if act_heavy:
        nc.scalar.activation(mu[:, :], p_s[:, :], FT.Copy, scale=1.0 / E)
        nc.scalar.activation(mu2[:, :], mu[:, :], FT.Square)
    else:
        nc.vector.tensor_scalar_mul(mu[:, :], p_s[:, :], 1.0 / E)
        nc.vector.tensor_mul(mu2[:, :], mu[:, :], mu[:, :])
    var = ps.tile([128, wn], f32, name="lnvar", tag="lnvar")
    nc.vector.scalar_tensor_tensor(var[:, :], p_sq[:, :], 1.0 / E, mu2[:, :],
                                   ALU.mult, ALU.subtract)
    sd = ps.tile([128, wn], f32, name="lnsd", tag="lnsd")
    nc.scalar.activation(sd[:, :], var[:, :], FT.Sqrt, bias=eps_ap)
    rstd = ps.tile([128, wn], f32, name="lnrstd", tag="lnrstd")
    nc.vector.reciprocal_approx_fast(rstd[:, :], sd[:, :])
    for c in range(EC):
        xs = x_chunks[c][:, win:win + wn]
        xm = ps.tile([128, wn], f32, name="lnxm", tag="lnxm")
        nc.vector.tensor_sub(xm[:, :], xs, mu[:, :])
        xm2 = ps.tile([128, wn], f32, name="lnxm2", tag="lnxm2")
        nc.vector.tensor_mul(xm2[:, :], xm[:, :], rstd[:, :])
        nc.vector.tensor_scalar(
            out_chunks[c][:, win:win + wn], xm2[:, :],
            vecs[:, 2 * g_col + c:2 * g_col + c + 1],
            vecs[:, 2 * beta_col + c:2 * beta_col + c + 1],
            ALU.mult, ALU.add)


def build_body(nc, tc, xT_d, gabT_d, w_d, vecs_d, ident_d, outT_d):
    persist_pools = []

    def ppool(name):
        p = tc.tile_pool(name=name, bufs=1)
        persist_pools.append(p)
        return p.__enter__()

    persist = ppool("persist")

    # ---- persistent SBUF ----
    qt = [persist.tile([128, NQ], bf16, name=f"qt{c}", tag=f"qt{c}")
          for c in range(EC)]
    kt = [persist.tile([128, N], bf16, name=f"kt{c}", tag=f"kt{c}")
          for c in range(EC)]
    xt = [persist.tile([128, N], f32, name=f"xt{c}", tag=f"xt{c}")
          for c in range(EC)]
    v_sb = [persist.tile([128, E], bf16, name=f"v{k}", tag=f"v{k}") for k in range(NKC)]
    gab_sb = [persist.tile([128, 2 * NQ], bf16, name=f"gab_{k}", tag=f"gab_{k}")
              for k in range(NKC)]
    w_sb = {n: persist.tile([128, 2 * E], f32, name=f"w_{n}", tag=f"w_{n}") for n in w_d}
    w_bf = {n: persist.tile([128, 2 * E], bf16, name=f"wbf_{n}", tag=f"wbf_{n}")
            for n in w_d}
    vecs = persist.tile([128, 14], f32, name="vecs", tag="vecs")
    id_bf = persist.tile([128, 128], bf16, name="id_bf", tag="id_bf")
    ones = persist.tile([128, 128], f32, name="ones", tag="ones")
    ones_bf = persist.tile([128, 32], bf16, name="ones_bf", tag="ones_bf")
    h_sb = [[persist.tile([128, QC], f32, name=f"h{q}{c}", tag=f"h{q}{c}") for c in range(EC)]
            for q in range(2)]

    # ---- small loads ----
    for n in w_d:
        for c in range(EC):
            nc.sync.dma_start(w_sb[n][:, E * c:E * (c + 1)],
                              w_d[n][128 * c:128 * (c + 1), :])
    nc.sync.dma_start(vecs[:, :], vecs_d[:, :])
    idt = persist.tile([128, 128], f32, name="id_f32", tag="id_f32")
    nc.sync.dma_start(idt[:, :], ident_d[:, :])
    nc.vector.tensor_copy(id_bf[:, :], idt[:, :])
    nc.vector.memset(ones[:, :], 1.0)
    nc.vector.memset(ones_bf[:, :], 1.0)
    eps_t = persist.tile([128, 1], f32, name="eps_t", tag="eps_t")
    nc.vector.memset(eps_t[:, :], 1e-5)
    for n in w_d:
        nc.vector.tensor_copy(w_bf[n][:, :], w_sb[n][:, :])

    def emit_prep(kc):
        """Influence gates for chunk kc: one combined [EG | EG*G2] load."""
        nc.sync.dma_start(gab_sb[kc][:, :], gabT_d[128 * kc:128 * (kc + 1), :])

    # ---- stage B/C: LN1 + projections ----
    with tc.tile_pool(name="ln_pool", bufs=1) as lnp, \
         tc.tile_pool(name="ln_psum", bufs=2, space="PSUM") as ln_pp, \
         tc.tile_pool(name="ln_sbuf", bufs=2) as ln_ps, \
         tc.tile_pool(name="proj_psum", bufs=2, space="PSUM") as proj_psum:
        ln1 = [lnp.tile([128, N], bf16, name=f"ln1{c}", tag=f"ln1{c}") for c in range(EC)]
        for c in range(EC):
            nc.sync.dma_start(xt[c][:, :], xT_d[128 * c:128 * (c + 1), :])
        # gate loads queue behind xt/weights; they land well before use
        for kc in range(NKC):
            emit_prep(kc)
        for w in range(N // 512):
            layer_norm_T(nc, ln_pp, ln_ps, xt, 512 * w, 512, V_G1, V_BETA1,
                         vecs, ones, eps_t[:, :], ln1, act_heavy=False)
        for fc in range(EC):
            for qw in range(NQ // 512):
                pq = proj_psum.tile([128, 512], f32, name="proj", tag="proj")
                for ec in range(EC):
                    nc.tensor.matmul(
                        pq[:, :],
                        w_bf["Wq"][:, E * ec + 128 * fc:E * ec + 128 * (fc + 1)],
                        ln1[ec][:, 512 * qw:512 * (qw + 1)],
                        start=(ec == 0), stop=(ec == EC - 1))
                nc.vector.tensor_copy(qt[fc][:, 512 * qw:512 * (qw + 1)], pq[:, :])
        for fc in range(EC):
            for kw in range(N // 512):
                pk = proj_psum.tile([128, 512], f32, name="proj", tag="proj")
                for ec in range(EC):
                    nc.tensor.matmul(
                        pk[:, :],
                        w_bf["Wk"][:, E * ec + 128 * fc:E * ec + 128 * (fc + 1)],
                        ln1[ec][:, 512 * kw:512 * (kw + 1)],
                        start=(ec == 0), stop=(ec == EC - 1))
                nc.vector.tensor_copy(kt[fc][:, 512 * kw:512 * (kw + 1)], pk[:, :])
        for k in range(NKC):
            pv = proj_psum.tile([128, E], f32, name="projv", tag="projv")
            for ec in range(EC):
                nc.tensor.matmul(
                    pv[:, :],
                    ln1[ec][:, 128 * k:128 * (k + 1)],
                    w_bf["Wv"][:, E * ec:E * (ec + 1)],
                    start=(ec == 0), stop=(ec == EC - 1))
            nc.vector.tensor_copy(v_sb[k][:, :], pv[:, :])

    # ---- stage E: attention (software pipelined) + F: FFN ----
    _cms = []

    def mpool(name, bufs, space="SBUF"):
        cm = tc.tile_pool(name=name, bufs=bufs, space=space)
        _cms.append(cm)
        return cm.__enter__()

    sp = mpool("score_psum", 2, "PSUM")
    ap_ = mpool("acc_psum", 1, "PSUM")
    efp = mpool("ef_sbuf", 8)
    asb = mpool("att_sbuf", 2)

    acc = {}  # qc -> (wv_ps, z_ps)

    def emit_qk(qc, kc, half):
        """PE part 1 of iteration (qc, kc, half): 4 QK matmuls for heads
        4*half .. 4*half+3 (row-packed, start=True)."""
        q0 = QC * qc
        pre = is_preload(kc)
        sts = []
        for s in range(2):
            st = sp.tile([128, 2 * QC], f32, name="score", tag="score")
            sts.append(st)
        if pre:
            for st in sts:
                for j in range(2):
                    nc.tensor.matmul(
                        st[:, QC * j:QC * (j + 1)],
                        id_bf[:, :],
                        ga_sb[kc][:, q0:q0 + QC],
                        start=True, stop=False)
        for s in range(2):
            st = sts[s]
            for j in range(2):
                h = 4 * half + 2 * s + j
                c, hh = h // 4, 32 * (h % 4)
                nc.tensor.matmul(
                    st[:, QC * j:QC * (j + 1)],
                    kt[c][hh:hh + 32, 128 * kc:128 * (kc + 1)],
                    qt[c][hh:hh + 32, q0:q0 + QC],
                    start=not pre, stop=True,
                    skip_group_check=True, tile_position=(hh, 0))
        return (qc, kc, half, sts)

    def emit_rest(state):
        """ACT exp, DVE/GpSimd multiplies, PE Z/WV accumulation."""
        qc, kc, half, sts = state
        q0 = QC * qc
        wv_ps, z_ps = acc[qc]
        # gate view [p, g2, j2(bcast), q]: g=0 -> EG (zsrc), g=1 -> EG*G2 (f)
        g4 = gab_sb[kc][:, :].rearrange("p (g n) -> p g n", g=2)[
            :, :, q0:q0 + QC].rearrange(
            "p g (o q) -> p g o q", o=1).broadcast_to([128, 2, 2, QC])
        for s in range(2):
            st = sts[s]
            e = efp.tile([128, 2 * QC], bf16, name="e", tag="e")
            nc.scalar.activation(e[:, :], st[:, :], FT.Exp)
            e4 = e[:, :].rearrange(
                "p (o j q) -> p o j q", o=1, j=2).broadcast_to([128, 2, 2, QC])
            zf = efp.tile([128, 4 * QC], bf16, name="zf", tag="zf")
            nc.vector.tensor_tensor(
                zf[:, :].rearrange("p (g j q) -> p g j q", g=2, j=2),
                e4, g4, ALU.mult)
            for j in range(2):
                h = 4 * half + 2 * s + j
                s_, hh = h // 4, 32 * (h % 4)
                nc.tensor.matmul(
                    z_ps[s_][hh:hh + 32, :],
                    ones_bf[:, :],
                    zf[:, QC * j:QC * (j + 1)],
                    start=(kc == 0), stop=(kc == NKC - 1),
                    skip_group_check=True, tile_position=(0, hh))
            for j in range(2):
                h = 4 * half + 2 * s + j
                s_, hh = h // 4, 32 * (h % 4)
                nc.tensor.matmul(
                    wv_ps[s_][hh:hh + 32, :],
                    v_sb[kc][:, 32 * h:32 * h + 32],
                    zf[:, 2 * QC + QC * j:2 * QC + QC * (j + 1)],
                    start=(kc == 0), stop=(kc == NKC - 1),
                    skip_group_check=True, tile_position=(0, hh))

    def emit_attn_tail(qc):
        """normalize + Wo projection + bias + residual -> h_sb[qc]."""
        q0 = QC * qc
        wv_ps, z_ps = acc[qc]
        on = []
        for s in range(2):
            zr = asb.tile([128, QC], f32, name=f"zr{s}", tag=f"zr{s}")
            nc.vector.reciprocal_approx_fast(zr[:, :], z_ps[s][:, :])
            o = asb.tile([128, QC], bf16, name=f"on{s}", tag=f"on{s}")
            nc.vector.tensor_mul(o[:, :], wv_ps[s][:, :], zr[:, :])
            on.append(o)
        for fc in range(EC):
            po = sp.tile([128, QC], f32, name="score", tag="score")
            for ec in range(EC):
                nc.tensor.matmul(
                    po[:, :],
                    w_bf["Wo"][:, E * ec + 128 * fc:E * ec + 128 * (fc + 1)],
                    on[ec][:, :],
                    start=(ec == 0), stop=(ec == EC - 1))
            ta = asb.tile([128, QC], f32, name="tattn", tag="tattn")
            nc.scalar.activation(ta[:, :], po[:, :], FT.Identity,
                                 bias=vecs[:, 2 * V_BO + fc:2 * V_BO + fc + 1])
            nc.vector.tensor_add(h_sb[qc][fc][:, :], ta[:, :],
                                 xt[fc][:, q0:q0 + QC])

    def emit_ffn(qc, ln_pp2, ln_ps2, fp_, fs):
        """LN2 + FFN + residual + store for one q half."""
        ln2 = [fs.tile([128, QC], bf16, name=f"ln2{c}", tag=f"ln2{c}") for c in range(EC)]
        layer_norm_T(nc, ln_pp2, ln_ps2, h_sb[qc], 0, QC, V_G2, V_BETA2,
                     vecs, ones, eps_t[:, :], ln2)
        z1 = [fs.tile([128, QC], bf16, name=f"z1{c}", tag=f"z1{c}") for c in range(EC)]
        for fc in range(EC):
            p1 = fp_.tile([128, QC], f32, name="ffn", tag="ffn")
            for ec in range(EC):
                nc.tensor.matmul(
                    p1[:, :],
                    w_bf["W1"][:, E * ec + 128 * fc:E * ec + 128 * (fc + 1)],
                    ln2[ec][:, :],
                    start=(ec == 0), stop=(ec == EC - 1))
            nc.scalar.activation(z1[fc][:, :], p1[:, :], FT.Relu,
                                 bias=vecs[:, 2 * V_B1 + fc:2 * V_B1 + fc + 1])
        for fc in range(EC):
            p2 = fp_.tile([128, QC], f32, name="ffn", tag="ffn")
            for ec in range(EC):
                nc.tensor.matmul(
                    p2[:, :],
                    w_bf["W2"][:, E * ec + 128 * fc:E * ec + 128 * (fc + 1)],
                    z1[ec][:, :],
                    start=(ec == 0), stop=(ec == EC - 1))
            t2 = fs.tile([128, QC], f32, name="t2", tag="t2")
            nc.scalar.activation(t2[:, :], p2[:, :], FT.Identity,
                                 bias=vecs[:, 2 * V_B2 + fc:2 * V_B2 + fc + 1])
            of = fs.tile([128, QC], f32, name="of", tag="of")
            nc.gpsimd.tensor_add(of[:, :], t2[:, :], h_sb[qc][fc][:, :])
            nc.sync.dma_start(
                outT_d[128 * fc:128 * (fc + 1), QC * qc:QC * (qc + 1)],
                of[:, :])

    # ---- main software-pipelined loop ----
    for qc in range(2):
        wv_ps = [ap_.tile([128, QC], f32, name=f"wv{s}", tag=f"wv{s}") for s in range(2)]
        z_ps = [ap_.tile([128, QC], f32, name=f"z{s}", tag=f"z{s}") for s in range(2)]
        acc[qc] = (wv_ps, z_ps)
        pend = None
        for kc in range(NKC):
            for half in range(2):
                cur = emit_qk(qc, kc, half)
                if pend is not None:
                    emit_rest(pend)
                pend = cur
        emit_rest(pend)
        emit_attn_tail(qc)

    for cm in reversed(_cms):
        cm.__exit__(None, None, None)

    # ---- stage F: LN2 + FFN + residual + store (own PSUM scope) ----
    with tc.tile_pool(name="ln_psum2", bufs=2, space="PSUM") as ln_pp2, \
         tc.tile_pool(name="ln_sbuf2", bufs=2) as ln_ps2, \
         tc.tile_pool(name="ffn_psum", bufs=2, space="PSUM") as fp_, \
         tc.tile_pool(name="ffn_sbuf", bufs=2) as fs:
        for qc in range(2):
            emit_ffn(qc, ln_pp2, ln_ps2, fp_, fs)

    for p in reversed(persist_pools):
        p.__exit__(None, None, None)


def build_nc():
    nc = bacc.Bacc(
        "TRN2",
        target_bir_lowering=False,
        debug=False,
        enable_asserts=False,
        num_devices=8,
    )
    xT_d = nc.dram_tensor("xT", [E, N], f32, kind="ExternalInput").ap()
    gabT_d = nc.dram_tensor("gabT", [N, 2 * NQ], bf16, kind="ExternalInput").ap()
    w_d = {
        name: nc.dram_tensor(name, [E, E], f32, kind="ExternalInput").ap()
        for name in ("Wq", "Wk", "Wv", "Wo", "W1", "W2")
    }
    vecs_d = nc.dram_tensor("vecs", [128, 14], f32, kind="ExternalInput").ap()
    ident_d = nc.dram_tensor("ident", [128, 128], f32, kind="ExternalInput").ap()
    outT_d = nc.dram_tensor("outT", [E, NQ], f32, kind="ExternalOutput").ap()

    with tile.TileContext(nc) as tc:
        build_body(nc, tc, xT_d, gabT_d, w_d, vecs_d, ident_d, outT_d)
    nc.compile()
    return nc


def host_shard(inputs):
    """Build the 8 per-core input maps (see module docstring for the roll)."""
    x = np.asarray(inputs["x"], np.float32)
    infl = np.asarray(inputs["influence_matrix"], np.float32)
    iw1 = np.float32(inputs["iw1"])
    ib1 = np.float32(inputs["ib1"])
    iw2 = np.float32(inputs["iw2"])
    ib2 = np.float32(inputs["ib2"])
    vec_list = ["g1", "beta1", "g2", "beta2", "bo", "b1", "b2"]
    vecs_np = np.empty((128, 14), np.float32)
    for vi, nm in enumerate(vec_list):
        v = np.asarray(inputs[nm], np.float32).reshape(E)
        vecs_np[:, 2 * vi] = v[:128]
        vecs_np[:, 2 * vi + 1] = v[128:]
    ws = {n: np.ascontiguousarray(np.asarray(inputs[n], np.float32))
          for n in ("Wq", "Wv", "Wk", "Wo", "W1", "W2")}
    ws["Wq"] = ws["Wq"] / math.sqrt(D)

    # influence gates (shared across heads): EG = exp(iw1*u+ib1),
    # GB = EG*(iw2*u+ib2); shipped in bf16 per-core slices.
    import ml_dtypes
    lg = iw1 * infl + ib1
    g2 = iw2 * infl + ib2
    eg = np.exp(lg, dtype=np.float32)
    egg2 = eg * g2
    # per-k-chunk parity: preload chunks ship (LG, G2); others (EG, EG*G2).
    # The k-chunk index on the device is along the FIRST axis of the
    # transposed [N(k), NQ] slice, i.e. the original column axis of infl —
    # which is rolled per core. Build full-size gate tensors per core below.
    eg_bf = eg.astype(ml_dtypes.bfloat16)
    gb_bf = egg2.astype(ml_dtypes.bfloat16)
    lg_bf = lg.astype(ml_dtypes.bfloat16)
    g2_bf = g2.astype(ml_dtypes.bfloat16)

    in_maps = []
    for core in range(8):
        b, qh = core // 2, core % 2
        qoff = qh * NQ
        xb = np.roll(x[b], -qoff, axis=0)          # [N, E], own rows first
        xT = np.ascontiguousarray(xb.T)            # [E, N]
        gaT = np.roll(eg_bf[b][qoff:qoff + NQ, :], -qoff, axis=1).T  # [N(k), NQ]
        gbT = np.roll(gb_bf[b][qoff:qoff + NQ, :], -qoff, axis=1).T
        gabT = np.ascontiguousarray(np.concatenate([gaT, gbT], axis=1))
        m = {"xT": xT, "gabT": gabT, "vecs": vecs_np,
             "ident": np.eye(128, dtype=np.float32)}
        m.update(ws)
        in_maps.append(m)
    return in_maps


_NC_CACHE = []


def kernel(**inputs):
    if not _NC_CACHE:
        _NC_CACHE.append(build_nc())
    nc = _NC_CACHE[0]
    in_maps = host_shard(inputs)
    res = run_bass_kernel_spmd(nc, in_maps, core_ids=list(range(8)))
    out = np.empty((B, N, E), np.float32)
    for core in range(8):
        b, qh = core // 2, core % 2
        out[b, qh * NQ:(qh + 1) * NQ, :] = np.asarray(
            res.results[core]["outT"], np.float32).T
    return out


# revision 17
# speedup vs baseline: 1.1934x; 1.1203x over previous
"""Graphormer layer on 8 TRN2 NeuronCores.

Sharding: core c handles batch b = c//2 and query-row half qh = c%2 (1024 q
rows). All compute is in transposed (feature-on-partition) layout; the host
pre-transposes x and the influence slices and transposes per-core outputs
back during the gather. Host also rolls the node axis per core so each core's
own q rows sit at columns [0, 1024) — the device program is identical across
cores (attention over all keys is permutation-invariant; the influence k axis
is rolled identically).

v3 design notes:
  - The influence add is handled entirely multiplicatively:
    exp(s + LG) = exp(s)*EG with EG = exp(iw1*u + ib1). The host ships
    gaT = EG and gbT = EG*(iw2*u + ib2) in bf16 (same DMA bytes as the raw
    fp32 influence), so the device does NO influence prep at all — the PE
    identity-preload path and the GpSimd prep of earlier versions are gone.
  - The attention inner loop is software-pipelined: the QK matmuls of
    iteration i+1 are issued on the PE queue before the Z/WV matmuls of
    iteration i, so the PE never waits on the ACT exp / DVE multiply chain.
  - A subset of the Z-source multiplies runs on the otherwise-idle GpSimd
    engine to relieve the DVE.
  - LayerNorm is rebalanced across ACT (Square/Sqrt/scale) and DVE; bias
    adds and ReLU in the attention tail and FFN run on ACT (Copy/Identity/
    Relu/Square live in every ACT table, so only Exp<->Sqrt transitions
    reload tables: ~3 reloads per kernel).

Per core:
  ln1T = LayerNorm_T(xT)              (partition-dim LN via ones-matmul sums)
  QT/KT/V projections (bf16)
  per (qc, kc, head-pair):
    psum = KT_h.T @ QT_h              (bf16, contract d=32, row-packed)
    e = exp(psum)                     (ACT -> SBUF bf16)
    zsrc = e * EG[kc]                 (DVE/GpSimd bf16)
    f = e * (EG*G2)[kc]               (DVE bf16)
    Z  += ones.T @ zsrc               (bf16, col-packed per head)
    wv += V_kc-slice.T @ f            (bf16, col-packed per head)
  attn = (wv / Z) @ Wo + bo ; h = attn + xT_own
  out = W2.T-proj(relu(W1.T-proj(LN_T(h)) + b1)) + b2 + h
"""

import math

import numpy as np

import concourse.bass as bass
import concourse.bacc as bacc
import concourse.mybir as mybir
import concourse.tile as tile
from concourse.bass_utils import run_bass_kernel_spmd

B, N, E, H, D = 4, 2048, 256, 8, 32
NQ = N // 2          # q rows per core
QC = 512             # q window
NKC = N // 128       # 16 k-chunks
EC = 2               # feature chunks

f32 = mybir.dt.float32
bf16 = mybir.dt.bfloat16
FT = mybir.ActivationFunctionType
ALU = mybir.AluOpType

# vecs_sb column index: vec v, chunk c -> 2*v + c
V_G1, V_BETA1, V_G2, V_BETA2, V_BO, V_B1, V_B2 = range(7)


def is_preload(kc):
    """kc chunks whose influence-add goes through a PSUM identity preload
    (LG/G2 shipped) instead of the multiplicative EG path."""
    return kc % 4 == 0


def layer_norm_T(nc, pp, ps, x_chunks, win, wn, g_col, beta_col, vecs, ones,
                 eps_ap, out_chunks, act_heavy=True):
    """LayerNorm over the partition dim (E = 2 chunks) in T layout.

    x_chunks: 2 SBUF APs; normalizes cols [win:win+wn] -> out_chunks.
    Square/scale on ACT, rstd = 1/sqrt via ACT Sqrt + DVE reciprocal.
    """
    sq = ps.tile([128, 2 * wn], f32, name="lnsq", tag="lnsq")
    p_s = pp.tile([128, wn], f32, name="lnps", tag="lnps")
    p_sq = pp.tile([128, wn], f32, name="lnpsq", tag="lnpsq")
    for c in range(EC):
        xs = x_chunks[c][:, win:win + wn]
        if act_heavy:
            nc.scalar.activation(sq[:, c * wn:(c + 1) * wn], xs, FT.Square)
        else:
            nc.vector.tensor_mul(sq[:, c * wn:(c + 1) * wn], xs, xs)
        nc.tensor.matmul(p_s[:, :], ones[:, :], xs,
                         start=(c == 0), stop=(c == EC - 1))
    for c in range(EC):
        nc.tensor.matmul(p_sq[:, :], ones[:, :],
                         sq[:, c * wn:(c + 1) * wn],
                         start=(c == 0), stop=(c == EC - 1))
    mu = ps.tile([128, wn], f32, name="lnmu", tag="lnmu")
    mu2 = ps.tile([128, wn], f32, name="lnmu2", tag="lnmu2")
    if act_heavy:
        nc.scalar.activation(mu[:, :], p_s[:, :], FT.Copy, scale=1.0 / E)
        nc.scalar.activation(mu2[:, :], mu[:, :], FT.Square)
    else:
        nc.vector.tensor_scalar_mul(mu[:, :], p_s[:, :], 1.0 / E)
        nc.vector.tensor_mul(mu2[:, :], mu[:, :], mu[:, :])
    var = ps.tile([128, wn], f32, name="lnvar", tag="lnvar")
    nc.vector.scalar_tensor_tensor(var[:, :], p_sq[:, :], 1.0 / E, mu2[:, :],
                                   ALU.mult, ALU.subtract)
    sd = ps.tile([128, wn], f32, name="lnsd", tag="lnsd")
    nc.scalar.activation(sd[:, :], var[:, :], FT.Sqrt, bias=eps_ap)
    rstd = ps.tile([128, wn], f32, name="lnrstd", tag="lnrstd")
    nc.vector.reciprocal_approx_fast(rstd[:, :], sd[:, :])
    xmeng = nc.gpsimd if act_heavy else nc.vector
    for c in range(EC):
        xs = x_chunks[c][:, win:win + wn]
        xm = ps.tile([128, wn], f32, name="lnxm", tag="lnxm")
        xmeng.tensor_sub(xm[:, :], xs, mu[:, :])
        xm2 = ps.tile([128, wn], f32, name="lnxm2", tag="lnxm2")
        xmeng.tensor_mul(xm2[:, :], xm[:, :], rstd[:, :])
        xmeng.tensor_scalar(
            out_chunks[c][:, win:win + wn], xm2[:, :],
            vecs[:, 2 * g_col + c:2 * g_col + c + 1],
            vecs[:, 2 * beta_col + c:2 * beta_col + c + 1],
            ALU.mult, ALU.add)


def build_body(nc, tc, xT_d, gabT_d, w_d, vecs_d, ident_d, outT_d):
    persist_pools = []

    def ppool(name):
        p = tc.tile_pool(name=name, bufs=1)
        persist_pools.append(p)
        return p.__enter__()

    persist = ppool("persist")

    # ---- persistent SBUF ----
    qt = [persist.tile([128, NQ], bf16, name=f"qt{c}", tag=f"qt{c}")
          for c in range(EC)]
    kt = [persist.tile([128, N], bf16, name=f"kt{c}", tag=f"kt{c}")
          for c in range(EC)]
    xt = [persist.tile([128, N], f32, name=f"xt{c}", tag=f"xt{c}")
          for c in range(EC)]
    v_sb = [persist.tile([128, E], bf16, name=f"v{k}", tag=f"v{k}") for k in range(NKC)]
    gab_sb = [persist.tile([128, 2 * NQ], bf16, name=f"gab_{k}", tag=f"gab_{k}")
              for k in range(NKC)]
    w_sb = {n: persist.tile([128, 2 * E], f32, name=f"w_{n}", tag=f"w_{n}") for n in w_d}
    w_bf = {n: persist.tile([128, 2 * E], bf16, name=f"wbf_{n}", tag=f"wbf_{n}")
            for n in w_d}
    vecs = persist.tile([128, 14], f32, name="vecs", tag="vecs")
    id_bf = persist.tile([128, 128], bf16, name="id_bf", tag="id_bf")
    ones = persist.tile([128, 128], f32, name="ones", tag="ones")
    ones_bf = persist.tile([128, 32], bf16, name="ones_bf", tag="ones_bf")
    h_sb = [[persist.tile([128, QC], f32, name=f"h{q}{c}", tag=f"h{q}{c}") for c in range(EC)]
            for q in range(2)]

    # ---- small loads ----
    for n in w_d:
        for c in range(EC):
            nc.sync.dma_start(w_sb[n][:, E * c:E * (c + 1)],
                              w_d[n][128 * c:128 * (c + 1), :])
    nc.sync.dma_start(vecs[:, :], vecs_d[:, :])
    idt = persist.tile([128, 128], f32, name="id_f32", tag="id_f32")
    nc.sync.dma_start(idt[:, :], ident_d[:, :])
    nc.vector.tensor_copy(id_bf[:, :], idt[:, :])
    nc.vector.memset(ones[:, :], 1.0)
    nc.vector.memset(ones_bf[:, :], 1.0)
    eps_t = persist.tile([128, 1], f32, name="eps_t", tag="eps_t")
    nc.vector.memset(eps_t[:, :], 1e-5)
    for n in w_d:
        nc.vector.tensor_copy(w_bf[n][:, :], w_sb[n][:, :])

    def emit_prep(kc):
        """Influence gates for chunk kc: one combined [EG | EG*G2] load."""
        nc.sync.dma_start(gab_sb[kc][:, :], gabT_d[128 * kc:128 * (kc + 1), :])

    # ---- stage B/C: LN1 + projections ----
    with tc.tile_pool(name="ln_pool", bufs=1) as lnp, \
         tc.tile_pool(name="ln_psum", bufs=2, space="PSUM") as ln_pp, \
         tc.tile_pool(name="ln_sbuf", bufs=2) as ln_ps, \
         tc.tile_pool(name="proj_psum", bufs=2, space="PSUM") as proj_psum:
        ln1 = [lnp.tile([128, N], bf16, name=f"ln1{c}", tag=f"ln1{c}") for c in range(EC)]
        for c in range(EC):
            nc.sync.dma_start(xt[c][:, :], xT_d[128 * c:128 * (c + 1), :])
        # gate loads queue behind xt/weights; they land well before use
        for kc in range(NKC):
            emit_prep(kc)
        for w in range(N // 512):
            layer_norm_T(nc, ln_pp, ln_ps, xt, 512 * w, 512, V_G1, V_BETA1,
                         vecs, ones, eps_t[:, :], ln1, act_heavy=False)
        for fc in range(EC):
            for qw in range(NQ // 512):
                pq = proj_psum.tile([128, 512], f32, name="proj", tag="proj")
                for ec in range(EC):
                    nc.tensor.matmul(
                        pq[:, :],
                        w_bf["Wq"][:, E * ec + 128 * fc:E * ec + 128 * (fc + 1)],
                        ln1[ec][:, 512 * qw:512 * (qw + 1)],
                        start=(ec == 0), stop=(ec == EC - 1))
                nc.vector.tensor_copy(qt[fc][:, 512 * qw:512 * (qw + 1)], pq[:, :])
        for fc in range(EC):
            for kw in range(N // 512):
                pk = proj_psum.tile([128, 512], f32, name="proj", tag="proj")
                for ec in range(EC):
                    nc.tensor.matmul(
                        pk[:, :],
                        w_bf["Wk"][:, E * ec + 128 * fc:E * ec + 128 * (fc + 1)],
                        ln1[ec][:, 512 * kw:512 * (kw + 1)],
                        start=(ec == 0), stop=(ec == EC - 1))
                nc.vector.tensor_copy(kt[fc][:, 512 * kw:512 * (kw + 1)], pk[:, :])
        for k in range(NKC):
            pv = proj_psum.tile([128, E], f32, name="projv", tag="projv")
            for ec in range(EC):
                nc.tensor.matmul(
                    pv[:, :],
                    ln1[ec][:, 128 * k:128 * (k + 1)],
                    w_bf["Wv"][:, E * ec:E * (ec + 1)],
                    start=(ec == 0), stop=(ec == EC - 1))
            nc.vector.tensor_copy(v_sb[k][:, :], pv[:, :])

    # ---- stage E: attention (software pipelined) + F: FFN ----
    _cms = []

    def mpool(name, bufs, space="SBUF"):
        cm = tc.tile_pool(name=name, bufs=bufs, space=space)
        _cms.append(cm)
        return cm.__enter__()

    sp = mpool("score_psum", 2, "PSUM")
    ap_ = mpool("acc_psum", 1, "PSUM")
    efp = mpool("ef_sbuf", 8)
    asb = mpool("att_sbuf", 2)

    acc = {}  # qc -> (wv_ps, z_ps)

    def emit_qk(qc, kc, half):
        """PE part 1 of iteration (qc, kc, half): 4 QK matmuls for heads
        4*half .. 4*half+3 (row-packed, start=True)."""
        q0 = QC * qc
        pre = is_preload(kc)
        sts = []
        for s in range(2):
            st = sp.tile([128, 2 * QC], f32, name="score", tag="score")
            sts.append(st)
        if pre:
            for st in sts:
                for j in range(2):
                    nc.tensor.matmul(
                        st[:, QC * j:QC * (j + 1)],
                        id_bf[:, :],
                        gab_sb[kc][:, q0:q0 + QC],
                        start=True, stop=False)
        for s in range(2):
            st = sts[s]
            for j in range(2):
                h = 4 * half + 2 * s + j
                c, hh = h // 4, 32 * (h % 4)
                nc.tensor.matmul(
                    st[:, QC * j:QC * (j + 1)],
                    kt[c][hh:hh + 32, 128 * kc:128 * (kc + 1)],
                    qt[c][hh:hh + 32, q0:q0 + QC],
                    start=not pre, stop=True,
                    skip_group_check=True, tile_position=(hh, 0))
        return (qc, kc, half, sts)

    def emit_rest(state):
        """ACT exp, DVE/GpSimd multiplies, PE Z/WV accumulation."""
        qc, kc, half, sts = state
        q0 = QC * qc
        wv_ps, z_ps = acc[qc]
        # gate view [p, g2, j2(bcast), q]: g=0 -> EG (zsrc), g=1 -> EG*G2 (f)
        g4 = gab_sb[kc][:, :].rearrange("p (g n) -> p g n", g=2)[
            :, :, q0:q0 + QC].rearrange(
            "p g (o q) -> p g o q", o=1).broadcast_to([128, 2, 2, QC])
        pre = is_preload(kc)
        for s in range(2):
            st = sts[s]
            e = efp.tile([128, 2 * QC], bf16, name="e", tag="e")
            nc.scalar.activation(e[:, :], st[:, :], FT.Exp)
            if pre:
                # e already carries the influence add; one f-mult only
                zsrc_ap = e
                zf = efp.tile([128, 2 * QC], bf16, name="zf", tag="zf")
                gb2 = gab_sb[kc][:, :].rearrange("p (g n) -> p g n", g=2)[
                    :, 1:2, q0:q0 + QC].broadcast_to([128, 2, QC])
                nc.vector.tensor_tensor(
                    zf[:, :].rearrange("p (j q) -> p j q", j=2),
                    e[:, :].rearrange("p (j q) -> p j q", j=2),
                    gb2, ALU.mult)
                f_off = 0
            else:
                e4 = e[:, :].rearrange(
                    "p (o j q) -> p o j q", o=1, j=2).broadcast_to([128, 2, 2, QC])
                zf = efp.tile([128, 4 * QC], bf16, name="zf", tag="zf")
                nc.vector.tensor_tensor(
                    zf[:, :].rearrange("p (g j q) -> p g j q", g=2, j=2),
                    e4, g4, ALU.mult)
                zsrc_ap = zf
                f_off = 2 * QC
            for j in range(2):
                h = 4 * half + 2 * s + j
                s_, hh = h // 4, 32 * (h % 4)
                nc.tensor.matmul(
                    z_ps[s_][hh:hh + 32, :],
                    ones_bf[:, :],
                    zsrc_ap[:, QC * j:QC * (j + 1)],
                    start=(kc == 0), stop=(kc == NKC - 1),
                    skip_group_check=True, tile_position=(0, hh))
            for j in range(2):
                h = 4 * half + 2 * s + j
                s_, hh = h // 4, 32 * (h % 4)
                nc.tensor.matmul(
                    wv_ps[s_][hh:hh + 32, :],
                    v_sb[kc][:, 32 * h:32 * h + 32],
                    zf[:, f_off + QC * j:f_off + QC * (j + 1)],
                    start=(kc == 0), stop=(kc == NKC - 1),
                    skip_group_check=True, tile_position=(0, hh))

    def emit_attn_tail(qc):
        """normalize + Wo projection + bias + residual -> h_sb[qc]."""
        q0 = QC * qc
        wv_ps, z_ps = acc[qc]
        on = []
        for s in range(2):
            zr = asb.tile([128, QC], f32, name=f"zr{s}", tag=f"zr{s}")
            nc.vector.reciprocal_approx_fast(zr[:, :], z_ps[s][:, :])
            o = asb.tile([128, QC], bf16, name=f"on{s}", tag=f"on{s}")
            nc.vector.tensor_mul(o[:, :], wv_ps[s][:, :], zr[:, :])
            on.append(o)
        for fc in range(EC):
            po = sp.tile([128, QC], f32, name="score", tag="score")
            for ec in range(EC):
                nc.tensor.matmul(
                    po[:, :],
                    w_bf["Wo"][:, E * ec + 128 * fc:E * ec + 128 * (fc + 1)],
                    on[ec][:, :],
                    start=(ec == 0), stop=(ec == EC - 1))
            ta = asb.tile([128, QC], f32, name="tattn", tag="tattn")
            nc.scalar.activation(ta[:, :], po[:, :], FT.Identity,
                                 bias=vecs[:, 2 * V_BO + fc:2 * V_BO + fc + 1])
            nc.gpsimd.tensor_add(h_sb[qc][fc][:, :], ta[:, :],
                                  xt[fc][:, q0:q0 + QC])

    def emit_ffn(qc, ln_pp2, ln_ps2, fp_, fs):
        """LN2 + FFN + residual + store for one q half."""
        ln2 = [fs.tile([128, QC], bf16, name=f"ln2{c}", tag=f"ln2{c}") for c in range(EC)]
        layer_norm_T(nc, ln_pp2, ln_ps2, h_sb[qc], 0, QC, V_G2, V_BETA2,
                     vecs, ones, eps_t[:, :], ln2)
        z1 = [fs.tile([128, QC], bf16, name=f"z1{c}", tag=f"z1{c}") for c in range(EC)]
        for fc in range(EC):
            p1 = fp_.tile([128, QC], f32, name="ffn", tag="ffn")
            for ec in range(EC):
                nc.tensor.matmul(
                    p1[:, :],
                    w_bf["W1"][:, E * ec + 128 * fc:E * ec + 128 * (fc + 1)],
                    ln2[ec][:, :],
                    start=(ec == 0), stop=(ec == EC - 1))
            nc.scalar.activation(z1[fc][:, :], p1[:, :], FT.Relu,
                                 bias=vecs[:, 2 * V_B1 + fc:2 * V_B1 + fc + 1])
        for fc in range(EC):
            p2 = fp_.tile([128, QC], f32, name="ffn", tag="ffn")
            for ec in range(EC):
                nc.tensor.matmul(
                    p2[:, :],
                    w_bf["W2"][:, E * ec + 128 * fc:E * ec + 128 * (fc + 1)],
                    z1[ec][:, :],
                    start=(ec == 0), stop=(ec == EC - 1))
            t2 = fs.tile([128, QC], f32, name="t2", tag="t2")
            nc.scalar.activation(t2[:, :], p2[:, :], FT.Identity,
                                 bias=vecs[:, 2 * V_B2 + fc:2 * V_B2 + fc + 1])
            of = fs.tile([128, QC], f32, name="of", tag="of")
            nc.gpsimd.tensor_add(of[:, :], t2[:, :], h_sb[qc][fc][:, :])
            nc.sync.dma_start(
                outT_d[128 * fc:128 * (fc + 1), QC * qc:QC * (qc + 1)],
                of[:, :])

    # ---- main software-pipelined loop ----
    for qc in range(2):
        wv_ps = [ap_.tile([128, QC], f32, name=f"wv{s}", tag=f"wv{s}") for s in range(2)]
        z_ps = [ap_.tile([128, QC], f32, name=f"z{s}", tag=f"z{s}") for s in range(2)]
        acc[qc] = (wv_ps, z_ps)
        pend = None
        for kc in range(NKC):
            for half in range(2):
                cur = emit_qk(qc, kc, half)
                if pend is not None:
                    emit_rest(pend)
                pend = cur
        emit_rest(pend)
        emit_attn_tail(qc)

    for cm in reversed(_cms):
        cm.__exit__(None, None, None)

    # ---- stage F: LN2 + FFN + residual + store (own PSUM scope) ----
    with tc.tile_pool(name="ln_psum2", bufs=2, space="PSUM") as ln_pp2, \
         tc.tile_pool(name="ln_sbuf2", bufs=2) as ln_ps2, \
         tc.tile_pool(name="ffn_psum", bufs=2, space="PSUM") as fp_, \
         tc.tile_pool(name="ffn_sbuf", bufs=2) as fs:
        for qc in range(2):
            emit_ffn(qc, ln_pp2, ln_ps2, fp_, fs)

    for p in reversed(persist_pools):
        p.__exit__(None, None, None)


def build_nc():
    nc = bacc.Bacc(
        "TRN2",
        target_bir_lowering=False,
        debug=False,
        enable_asserts=False,
        num_devices=8,
    )
    xT_d = nc.dram_tensor("xT", [E, N], f32, kind="ExternalInput").ap()
    gabT_d = nc.dram_tensor("gabT", [N, 2 * NQ], bf16, kind="ExternalInput").ap()
    w_d = {
        name: nc.dram_tensor(name, [E, E], f32, kind="ExternalInput").ap()
        for name in ("Wq", "Wk", "Wv", "Wo", "W1", "W2")
    }
    vecs_d = nc.dram_tensor("vecs", [128, 14], f32, kind="ExternalInput").ap()
    ident_d = nc.dram_tensor("ident", [128, 128], f32, kind="ExternalInput").ap()
    outT_d = nc.dram_tensor("outT", [E, NQ], f32, kind="ExternalOutput").ap()

    with tile.TileContext(nc) as tc:
        build_body(nc, tc, xT_d, gabT_d, w_d, vecs_d, ident_d, outT_d)
    nc.compile()
    return nc


def host_shard(inputs):
    """Build the 8 per-core input maps (see module docstring for the roll)."""
    x = np.asarray(inputs["x"], np.float32)
    infl = np.asarray(inputs["influence_matrix"], np.float32)
    iw1 = np.float32(inputs["iw1"])
    ib1 = np.float32(inputs["ib1"])
    iw2 = np.float32(inputs["iw2"])
    ib2 = np.float32(inputs["ib2"])
    vec_list = ["g1", "beta1", "g2", "beta2", "bo", "b1", "b2"]
    vecs_np = np.empty((128, 14), np.float32)
    for vi, nm in enumerate(vec_list):
        v = np.asarray(inputs[nm], np.float32).reshape(E)
        vecs_np[:, 2 * vi] = v[:128]
        vecs_np[:, 2 * vi + 1] = v[128:]
    ws = {n: np.ascontiguousarray(np.asarray(inputs[n], np.float32))
          for n in ("Wq", "Wv", "Wk", "Wo", "W1", "W2")}
    ws["Wq"] = ws["Wq"] / math.sqrt(D)

    # influence gates (shared across heads): EG = exp(iw1*u+ib1),
    # GB = EG*(iw2*u+ib2); shipped in bf16 per-core slices.
    import ml_dtypes
    lg = iw1 * infl + ib1
    g2 = iw2 * infl + ib2
    eg = np.exp(lg, dtype=np.float32)
    egg2 = eg * g2
    # per-k-chunk parity: preload chunks ship (LG, G2); others (EG, EG*G2).
    # The k-chunk index on the device is along the FIRST axis of the
    # transposed [N(k), NQ] slice, i.e. the original column axis of infl —
    # which is rolled per core. Build full-size gate tensors per core below.
    eg_bf = eg.astype(ml_dtypes.bfloat16)
    gb_bf = egg2.astype(ml_dtypes.bfloat16)
    lg_bf = lg.astype(ml_dtypes.bfloat16)
    g2_bf = g2.astype(ml_dtypes.bfloat16)

    in_maps = []
    for core in range(8):
        b, qh = core // 2, core % 2
        qoff = qh * NQ
        xb = np.roll(x[b], -qoff, axis=0)          # [N, E], own rows first
        xT = np.ascontiguousarray(xb.T)            # [E, N]
        gaT = np.roll(eg_bf[b][qoff:qoff + NQ, :], -qoff, axis=1).T  # [N(k), NQ]
        gbT = np.roll(gb_bf[b][qoff:qoff + NQ, :], -qoff, axis=1).T
        lgT = np.roll(lg_bf[b][qoff:qoff + NQ, :], -qoff, axis=1).T
        g2T = np.roll(g2_bf[b][qoff:qoff + NQ, :], -qoff, axis=1).T
        gabT = np.concatenate([gaT, gbT], axis=1)
        for kc in range(NKC):
            if kc % 4 == 0:   # must match is_preload()
                sl = slice(128 * kc, 128 * (kc + 1))
                gabT[sl, :NQ] = lgT[sl]
                gabT[sl, NQ:] = g2T[sl]
        gabT = np.ascontiguousarray(gabT)
        m = {"xT": xT, "gabT": gabT, "vecs": vecs_np,
             "ident": np.eye(128, dtype=np.float32)}
        m.update(ws)
        in_maps.append(m)
    return in_maps


_NC_CACHE = []


def kernel(**inputs):
    if not _NC_CACHE:
        _NC_CACHE.append(build_nc())
    nc = _NC_CACHE[0]
    in_maps = host_shard(inputs)
    res = run_bass_kernel_spmd(nc, in_maps, core_ids=list(range(8)))
    out = np.empty((B, N, E), np.float32)
    for core in range(8):
        b, qh = core // 2, core % 2
        out[b, qh * NQ:(qh + 1) * NQ, :] = np.asarray(
            res.results[core]["outT"], np.float32).T
    return out


# revision 18
# speedup vs baseline: 1.1996x; 1.0052x over previous
"""Graphormer layer on 8 TRN2 NeuronCores.

Sharding: core c handles batch b = c//2 and query-row half qh = c%2 (1024 q
rows). All compute is in transposed (feature-on-partition) layout; the host
pre-transposes x and the influence slices and transposes per-core outputs
back during the gather. Host also rolls the node axis per core so each core's
own q rows sit at columns [0, 1024) — the device program is identical across
cores (attention over all keys is permutation-invariant; the influence k axis
is rolled identically).

v3 design notes:
  - The influence add is handled entirely multiplicatively:
    exp(s + LG) = exp(s)*EG with EG = exp(iw1*u + ib1). The host ships
    gaT = EG and gbT = EG*(iw2*u + ib2) in bf16 (same DMA bytes as the raw
    fp32 influence), so the device does NO influence prep at all — the PE
    identity-preload path and the GpSimd prep of earlier versions are gone.
  - The attention inner loop is software-pipelined: the QK matmuls of
    iteration i+1 are issued on the PE queue before the Z/WV matmuls of
    iteration i, so the PE never waits on the ACT exp / DVE multiply chain.
  - A subset of the Z-source multiplies runs on the otherwise-idle GpSimd
    engine to relieve the DVE.
  - LayerNorm is rebalanced across ACT (Square/Sqrt/scale) and DVE; bias
    adds and ReLU in the attention tail and FFN run on ACT (Copy/Identity/
    Relu/Square live in every ACT table, so only Exp<->Sqrt transitions
    reload tables: ~3 reloads per kernel).

Per core:
  ln1T = LayerNorm_T(xT)              (partition-dim LN via ones-matmul sums)
  QT/KT/V projections (bf16)
  per (qc, kc, head-pair):
    psum = KT_h.T @ QT_h              (bf16, contract d=32, row-packed)
    e = exp(psum)                     (ACT -> SBUF bf16)
    zsrc = e * EG[kc]                 (DVE/GpSimd bf16)
    f = e * (EG*G2)[kc]               (DVE bf16)
    Z  += ones.T @ zsrc               (bf16, col-packed per head)
    wv += V_kc-slice.T @ f            (bf16, col-packed per head)
  attn = (wv / Z) @ Wo + bo ; h = attn + xT_own
  out = W2.T-proj(relu(W1.T-proj(LN_T(h)) + b1)) + b2 + h
"""

import math

import numpy as np

import concourse.bass as bass
import concourse.bacc as bacc
import concourse.mybir as mybir
import concourse.tile as tile
from concourse.bass_utils import run_bass_kernel_spmd

B, N, E, H, D = 4, 2048, 256, 8, 32
NQ = N // 2          # q rows per core
QC = 512             # q window
NKC = N // 128       # 16 k-chunks
EC = 2               # feature chunks

f32 = mybir.dt.float32
bf16 = mybir.dt.bfloat16
FT = mybir.ActivationFunctionType
ALU = mybir.AluOpType

# vecs_sb column index: vec v, chunk c -> 2*v + c
V_G1, V_BETA1, V_G2, V_BETA2, V_BO, V_B1, V_B2 = range(7)


def is_preload(kc):
    """kc chunks whose influence-add goes through a PSUM identity preload
    (LG/G2 shipped) instead of the multiplicative EG path."""
    return kc % 4 == 0


def layer_norm_T(nc, pp, ps, x_chunks, win, wn, g_col, beta_col, vecs, ones,
                 eps_ap, out_chunks, act_heavy=True):
    """LayerNorm over the partition dim (E = 2 chunks) in T layout.

    x_chunks: 2 SBUF APs; normalizes cols [win:win+wn] -> out_chunks.
    Square/scale on ACT, rstd = 1/sqrt via ACT Sqrt + DVE reciprocal.
    """
    sq = ps.tile([128, 2 * wn], f32, name="lnsq", tag="lnsq")
    p_s = pp.tile([128, wn], f32, name="lnps", tag="lnps")
    p_sq = pp.tile([128, wn], f32, name="lnpsq", tag="lnpsq")
    for c in range(EC):
        xs = x_chunks[c][:, win:win + wn]
        if act_heavy:
            nc.scalar.activation(sq[:, c * wn:(c + 1) * wn], xs, FT.Square)
        else:
            nc.vector.tensor_mul(sq[:, c * wn:(c + 1) * wn], xs, xs)
        nc.tensor.matmul(p_s[:, :], ones[:, :], xs,
                         start=(c == 0), stop=(c == EC - 1))
    for c in range(EC):
        nc.tensor.matmul(p_sq[:, :], ones[:, :],
                         sq[:, c * wn:(c + 1) * wn],
                         start=(c == 0), stop=(c == EC - 1))
    mu = ps.tile([128, wn], f32, name="lnmu", tag="lnmu")
    mu2 = ps.tile([128, wn], f32, name="lnmu2", tag="lnmu2")
    if act_heavy:
        nc.scalar.activation(mu[:, :], p_s[:, :], FT.Copy, scale=1.0 / E)
        nc.scalar.activation(mu2[:, :], mu[:, :], FT.Square)
    else:
        nc.vector.tensor_scalar_mul(mu[:, :], p_s[:, :], 1.0 / E)
        nc.vector.tensor_mul(mu2[:, :], mu[:, :], mu[:, :])
    var = ps.tile([128, wn], f32, name="lnvar", tag="lnvar")
    nc.vector.scalar_tensor_tensor(var[:, :], p_sq[:, :], 1.0 / E, mu2[:, :],
                                   ALU.mult, ALU.subtract)
    sd = ps.tile([128, wn], f32, name="lnsd", tag="lnsd")
    nc.scalar.activation(sd[:, :], var[:, :], FT.Sqrt, bias=eps_ap)
    rstd = ps.tile([128, wn], f32, name="lnrstd", tag="lnrstd")
    nc.vector.reciprocal_approx_fast(rstd[:, :], sd[:, :])
    xmeng = nc.vector
    for c in range(EC):
        xs = x_chunks[c][:, win:win + wn]
        xm = ps.tile([128, wn], f32, name="lnxm", tag="lnxm")
        xmeng.tensor_sub(xm[:, :], xs, mu[:, :])
        xm2 = ps.tile([128, wn], f32, name="lnxm2", tag="lnxm2")
        xmeng.tensor_mul(xm2[:, :], xm[:, :], rstd[:, :])
        xmeng.tensor_scalar(
            out_chunks[c][:, win:win + wn], xm2[:, :],
            vecs[:, 2 * g_col + c:2 * g_col + c + 1],
            vecs[:, 2 * beta_col + c:2 * beta_col + c + 1],
            ALU.mult, ALU.add)


def build_body(nc, tc, xT_d, gabT_d, w_d, vecs_d, ident_d, outT_d):
    persist_pools = []

    def ppool(name):
        p = tc.tile_pool(name=name, bufs=1)
        persist_pools.append(p)
        return p.__enter__()

    persist = ppool("persist")

    # ---- persistent SBUF ----
    qt = [persist.tile([128, NQ], bf16, name=f"qt{c}", tag=f"qt{c}")
          for c in range(EC)]
    kt = [persist.tile([128, N], bf16, name=f"kt{c}", tag=f"kt{c}")
          for c in range(EC)]
    xt = [persist.tile([128, N], f32, name=f"xt{c}", tag=f"xt{c}")
          for c in range(EC)]
    v_sb = [persist.tile([128, E], bf16, name=f"v{k}", tag=f"v{k}") for k in range(NKC)]
    gab_sb = [persist.tile([128, 2 * NQ], bf16, name=f"gab_{k}", tag=f"gab_{k}")
              for k in range(NKC)]
    w_sb = {n: persist.tile([128, 2 * E], f32, name=f"w_{n}", tag=f"w_{n}") for n in w_d}
    w_bf = {n: persist.tile([128, 2 * E], bf16, name=f"wbf_{n}", tag=f"wbf_{n}")
            for n in w_d}
    vecs = persist.tile([128, 14], f32, name="vecs", tag="vecs")
    id_bf = persist.tile([128, 128], bf16, name="id_bf", tag="id_bf")
    ones = persist.tile([128, 128], f32, name="ones", tag="ones")
    ones_bf = persist.tile([128, 32], bf16, name="ones_bf", tag="ones_bf")
    h_sb = [[persist.tile([128, QC], f32, name=f"h{q}{c}", tag=f"h{q}{c}") for c in range(EC)]
            for q in range(2)]

    # ---- small loads (xt first: LN1 gates on it) ----
    for c in range(EC):
        nc.sync.dma_start(xt[c][:, :], xT_d[128 * c:128 * (c + 1), :])
    for n in w_d:
        for c in range(EC):
            nc.sync.dma_start(w_sb[n][:, E * c:E * (c + 1)],
                              w_d[n][128 * c:128 * (c + 1), :])
    nc.sync.dma_start(vecs[:, :], vecs_d[:, :])
    idt = persist.tile([128, 128], f32, name="id_f32", tag="id_f32")
    nc.sync.dma_start(idt[:, :], ident_d[:, :])
    nc.vector.tensor_copy(id_bf[:, :], idt[:, :])
    nc.vector.memset(ones[:, :], 1.0)
    nc.vector.memset(ones_bf[:, :], 1.0)
    eps_t = persist.tile([128, 1], f32, name="eps_t", tag="eps_t")
    nc.vector.memset(eps_t[:, :], 1e-5)
    for n in w_d:
        nc.vector.tensor_copy(w_bf[n][:, :], w_sb[n][:, :])

    def emit_prep(kc):
        """Influence gates for chunk kc: one combined [EG | EG*G2] load.
        Issued on the ACT queue's HWDGE so it runs in parallel with the
        sync-queue xt/weight loads."""
        nc.scalar.dma_start(gab_sb[kc][:, :], gabT_d[128 * kc:128 * (kc + 1), :])

    # ---- stage B/C: LN1 + projections ----
    with tc.tile_pool(name="ln_pool", bufs=1) as lnp, \
         tc.tile_pool(name="ln_psum", bufs=2, space="PSUM") as ln_pp, \
         tc.tile_pool(name="ln_sbuf", bufs=2) as ln_ps, \
         tc.tile_pool(name="proj_psum", bufs=2, space="PSUM") as proj_psum:
        ln1 = [lnp.tile([128, N], bf16, name=f"ln1{c}", tag=f"ln1{c}") for c in range(EC)]
        for kc in range(NKC):
            emit_prep(kc)
        for w in range(N // 512):
            layer_norm_T(nc, ln_pp, ln_ps, xt, 512 * w, 512, V_G1, V_BETA1,
                         vecs, ones, eps_t[:, :], ln1, act_heavy=False)
        for fc in range(EC):
            for qw in range(NQ // 512):
                pq = proj_psum.tile([128, 512], f32, name="proj", tag="proj")
                for ec in range(EC):
                    nc.tensor.matmul(
                        pq[:, :],
                        w_bf["Wq"][:, E * ec + 128 * fc:E * ec + 128 * (fc + 1)],
                        ln1[ec][:, 512 * qw:512 * (qw + 1)],
                        start=(ec == 0), stop=(ec == EC - 1))
                nc.vector.tensor_copy(qt[fc][:, 512 * qw:512 * (qw + 1)], pq[:, :])
        for fc in range(EC):
            for kw in range(N // 512):
                pk = proj_psum.tile([128, 512], f32, name="proj", tag="proj")
                for ec in range(EC):
                    nc.tensor.matmul(
                        pk[:, :],
                        w_bf["Wk"][:, E * ec + 128 * fc:E * ec + 128 * (fc + 1)],
                        ln1[ec][:, 512 * kw:512 * (kw + 1)],
                        start=(ec == 0), stop=(ec == EC - 1))
                nc.vector.tensor_copy(kt[fc][:, 512 * kw:512 * (kw + 1)], pk[:, :])
        for k in range(NKC):
            pv = proj_psum.tile([128, E], f32, name="projv", tag="projv")
            for ec in range(EC):
                nc.tensor.matmul(
                    pv[:, :],
                    ln1[ec][:, 128 * k:128 * (k + 1)],
                    w_bf["Wv"][:, E * ec:E * (ec + 1)],
                    start=(ec == 0), stop=(ec == EC - 1))
            nc.vector.tensor_copy(v_sb[k][:, :], pv[:, :])

    # ---- stage E: attention (software pipelined) + F: FFN ----
    _cms = []

    def mpool(name, bufs, space="SBUF"):
        cm = tc.tile_pool(name=name, bufs=bufs, space=space)
        _cms.append(cm)
        return cm.__enter__()

    sp = mpool("score_psum", 2, "PSUM")
    ap_ = mpool("acc_psum", 1, "PSUM")
    efp = mpool("ef_sbuf", 8)
    asb = mpool("att_sbuf", 2)

    acc = {}  # qc -> (wv_ps, z_ps)

    def emit_qk(qc, kc, half):
        """PE part 1 of iteration (qc, kc, half): 4 QK matmuls for heads
        4*half .. 4*half+3 (row-packed, start=True)."""
        q0 = QC * qc
        pre = is_preload(kc)
        sts = []
        for s in range(2):
            st = sp.tile([128, 2 * QC], f32, name="score", tag="score")
            sts.append(st)
        if pre:
            for st in sts:
                for j in range(2):
                    nc.tensor.matmul(
                        st[:, QC * j:QC * (j + 1)],
                        id_bf[:, :],
                        gab_sb[kc][:, q0:q0 + QC],
                        start=True, stop=False)
        for s in range(2):
            st = sts[s]
            for j in range(2):
                h = 4 * half + 2 * s + j
                c, hh = h // 4, 32 * (h % 4)
                nc.tensor.matmul(
                    st[:, QC * j:QC * (j + 1)],
                    kt[c][hh:hh + 32, 128 * kc:128 * (kc + 1)],
                    qt[c][hh:hh + 32, q0:q0 + QC],
                    start=not pre, stop=True,
                    skip_group_check=True, tile_position=(hh, 0))
        return (qc, kc, half, sts)

    def emit_rest(state):
        """ACT exp, DVE/GpSimd multiplies, PE Z/WV accumulation."""
        qc, kc, half, sts = state
        q0 = QC * qc
        wv_ps, z_ps = acc[qc]
        # gate view [p, g2, j2(bcast), q]: g=0 -> EG (zsrc), g=1 -> EG*G2 (f)
        g4 = gab_sb[kc][:, :].rearrange("p (g n) -> p g n", g=2)[
            :, :, q0:q0 + QC].rearrange(
            "p g (o q) -> p g o q", o=1).broadcast_to([128, 2, 2, QC])
        pre = is_preload(kc)
        for s in range(2):
            st = sts[s]
            e = efp.tile([128, 2 * QC], bf16, name="e", tag="e")
            nc.scalar.activation(e[:, :], st[:, :], FT.Exp)
            if pre:
                # e already carries the influence add; one f-mult only
                zsrc_ap = e
                zf = efp.tile([128, 2 * QC], bf16, name="zf", tag="zf")
                gb2 = gab_sb[kc][:, :].rearrange("p (g n) -> p g n", g=2)[
                    :, 1:2, q0:q0 + QC].broadcast_to([128, 2, QC])
                nc.vector.tensor_tensor(
                    zf[:, :].rearrange("p (j q) -> p j q", j=2),
                    e[:, :].rearrange("p (j q) -> p j q", j=2),
                    gb2, ALU.mult)
                f_off = 0
            else:
                e4 = e[:, :].rearrange(
                    "p (o j q) -> p o j q", o=1, j=2).broadcast_to([128, 2, 2, QC])
                zf = efp.tile([128, 4 * QC], bf16, name="zf", tag="zf")
                nc.vector.tensor_tensor(
                    zf[:, :].rearrange("p (g j q) -> p g j q", g=2, j=2),
                    e4, g4, ALU.mult)
                zsrc_ap = zf
                f_off = 2 * QC
            for j in range(2):
                h = 4 * half + 2 * s + j
                s_, hh = h // 4, 32 * (h % 4)
                nc.tensor.matmul(
                    z_ps[s_][hh:hh + 32, :],
                    ones_bf[:, :],
                    zsrc_ap[:, QC * j:QC * (j + 1)],
                    start=(kc == 0), stop=(kc == NKC - 1),
                    skip_group_check=True, tile_position=(0, hh))
            for j in range(2):
                h = 4 * half + 2 * s + j
                s_, hh = h // 4, 32 * (h % 4)
                nc.tensor.matmul(
                    wv_ps[s_][hh:hh + 32, :],
                    v_sb[kc][:, 32 * h:32 * h + 32],
                    zf[:, f_off + QC * j:f_off + QC * (j + 1)],
                    start=(kc == 0), stop=(kc == NKC - 1),
                    skip_group_check=True, tile_position=(0, hh))

    def emit_attn_tail(qc):
        """normalize + Wo projection + bias + residual -> h_sb[qc]."""
        q0 = QC * qc
        wv_ps, z_ps = acc[qc]
        on = []
        for s in range(2):
            zr = asb.tile([128, QC], f32, name=f"zr{s}", tag=f"zr{s}")
            nc.vector.reciprocal_approx_fast(zr[:, :], z_ps[s][:, :])
            o = asb.tile([128, QC], bf16, name=f"on{s}", tag=f"on{s}")
            nc.vector.tensor_mul(o[:, :], wv_ps[s][:, :], zr[:, :])
            on.append(o)
        for fc in range(EC):
            po = sp.tile([128, QC], f32, name="score", tag="score")
            for ec in range(EC):
                nc.tensor.matmul(
                    po[:, :],
                    w_bf["Wo"][:, E * ec + 128 * fc:E * ec + 128 * (fc + 1)],
                    on[ec][:, :],
                    start=(ec == 0), stop=(ec == EC - 1))
            ta = asb.tile([128, QC], f32, name="tattn", tag="tattn")
            nc.scalar.activation(ta[:, :], po[:, :], FT.Identity,
                                 bias=vecs[:, 2 * V_BO + fc:2 * V_BO + fc + 1])
            nc.gpsimd.tensor_add(h_sb[qc][fc][:, :], ta[:, :],
                                  xt[fc][:, q0:q0 + QC])

    def emit_ffn(qc, ln_pp2, ln_ps2, fp_, fs):
        """LN2 + FFN + residual + store for one q half."""
        ln2 = [fs.tile([128, QC], bf16, name=f"ln2{c}", tag=f"ln2{c}") for c in range(EC)]
        layer_norm_T(nc, ln_pp2, ln_ps2, h_sb[qc], 0, QC, V_G2, V_BETA2,
                     vecs, ones, eps_t[:, :], ln2)
        z1 = [fs.tile([128, QC], bf16, name=f"z1{c}", tag=f"z1{c}") for c in range(EC)]
        for fc in range(EC):
            p1 = fp_.tile([128, QC], f32, name="ffn", tag="ffn")
            for ec in range(EC):
                nc.tensor.matmul(
                    p1[:, :],
                    w_bf["W1"][:, E * ec + 128 * fc:E * ec + 128 * (fc + 1)],
                    ln2[ec][:, :],
                    start=(ec == 0), stop=(ec == EC - 1))
            nc.scalar.activation(z1[fc][:, :], p1[:, :], FT.Relu,
                                 bias=vecs[:, 2 * V_B1 + fc:2 * V_B1 + fc + 1])
        for fc in range(EC):
            p2 = fp_.tile([128, QC], f32, name="ffn", tag="ffn")
            for ec in range(EC):
                nc.tensor.matmul(
                    p2[:, :],
                    w_bf["W2"][:, E * ec + 128 * fc:E * ec + 128 * (fc + 1)],
                    z1[ec][:, :],
                    start=(ec == 0), stop=(ec == EC - 1))
            t2 = fs.tile([128, QC], f32, name="t2", tag="t2")
            nc.scalar.activation(t2[:, :], p2[:, :], FT.Identity,
                                 bias=vecs[:, 2 * V_B2 + fc:2 * V_B2 + fc + 1])
            of = fs.tile([128, QC], f32, name="of", tag="of")
            nc.vector.tensor_add(of[:, :], t2[:, :], h_sb[qc][fc][:, :])
            nc.sync.dma_start(
                outT_d[128 * fc:128 * (fc + 1), QC * qc:QC * (qc + 1)],
                of[:, :])

    # ---- main software-pipelined loop ----
    for qc in range(2):
        wv_ps = [ap_.tile([128, QC], f32, name=f"wv{s}", tag=f"wv{s}") for s in range(2)]
        z_ps = [ap_.tile([128, QC], f32, name=f"z{s}", tag=f"z{s}") for s in range(2)]
        acc[qc] = (wv_ps, z_ps)
        pend = None
        for kc in range(NKC):
            for half in range(2):
                cur = emit_qk(qc, kc, half)
                if pend is not None:
                    emit_rest(pend)
                pend = cur
        emit_rest(pend)
        emit_attn_tail(qc)

    for cm in reversed(_cms):
        cm.__exit__(None, None, None)

    # ---- stage F: LN2 + FFN + residual + store (own PSUM scope) ----
    with tc.tile_pool(name="ln_psum2", bufs=2, space="PSUM") as ln_pp2, \
         tc.tile_pool(name="ln_sbuf2", bufs=2) as ln_ps2, \
         tc.tile_pool(name="ffn_psum", bufs=2, space="PSUM") as fp_, \
         tc.tile_pool(name="ffn_sbuf", bufs=2) as fs:
        for qc in range(2):
            emit_ffn(qc, ln_pp2, ln_ps2, fp_, fs)

    for p in reversed(persist_pools):
        p.__exit__(None, None, None)


def build_nc():
    nc = bacc.Bacc(
        "TRN2",
        target_bir_lowering=False,
        debug=False,
        enable_asserts=False,
        num_devices=8,
    )
    xT_d = nc.dram_tensor("xT", [E, N], f32, kind="ExternalInput").ap()
    gabT_d = nc.dram_tensor("gabT", [N, 2 * NQ], bf16, kind="ExternalInput").ap()
    w_d = {
        name: nc.dram_tensor(name, [E, E], f32, kind="ExternalInput").ap()
        for name in ("Wq", "Wk", "Wv", "Wo", "W1", "W2")
    }
    vecs_d = nc.dram_tensor("vecs", [128, 14], f32, kind="ExternalInput").ap()
    ident_d = nc.dram_tensor("ident", [128, 128], f32, kind="ExternalInput").ap()
    outT_d = nc.dram_tensor("outT", [E, NQ], f32, kind="ExternalOutput").ap()

    with tile.TileContext(nc) as tc:
        build_body(nc, tc, xT_d, gabT_d, w_d, vecs_d, ident_d, outT_d)
    nc.compile()
    return nc


def host_shard(inputs):
    """Build the 8 per-core input maps (see module docstring for the roll)."""
    x = np.asarray(inputs["x"], np.float32)
    infl = np.asarray(inputs["influence_matrix"], np.float32)
    iw1 = np.float32(inputs["iw1"])
    ib1 = np.float32(inputs["ib1"])
    iw2 = np.float32(inputs["iw2"])
    ib2 = np.float32(inputs["ib2"])
    vec_list = ["g1", "beta1", "g2", "beta2", "bo", "b1", "b2"]
    vecs_np = np.empty((128, 14), np.float32)
    for vi, nm in enumerate(vec_list):
        v = np.asarray(inputs[nm], np.float32).reshape(E)
        vecs_np[:, 2 * vi] = v[:128]
        vecs_np[:, 2 * vi + 1] = v[128:]
    ws = {n: np.ascontiguousarray(np.asarray(inputs[n], np.float32))
          for n in ("Wq", "Wv", "Wk", "Wo", "W1", "W2")}
    ws["Wq"] = ws["Wq"] / math.sqrt(D)

    # influence gates (shared across heads): EG = exp(iw1*u+ib1),
    # GB = EG*(iw2*u+ib2); shipped in bf16 per-core slices.
    import ml_dtypes
    lg = iw1 * infl + ib1
    g2 = iw2 * infl + ib2
    eg = np.exp(lg, dtype=np.float32)
    egg2 = eg * g2
    # per-k-chunk parity: preload chunks ship (LG, G2); others (EG, EG*G2).
    # The k-chunk index on the device is along the FIRST axis of the
    # transposed [N(k), NQ] slice, i.e. the original column axis of infl —
    # which is rolled per core. Build full-size gate tensors per core below.
    eg_bf = eg.astype(ml_dtypes.bfloat16)
    gb_bf = egg2.astype(ml_dtypes.bfloat16)
    lg_bf = lg.astype(ml_dtypes.bfloat16)
    g2_bf = g2.astype(ml_dtypes.bfloat16)

    in_maps = []
    for core in range(8):
        b, qh = core // 2, core % 2
        qoff = qh * NQ
        xb = np.roll(x[b], -qoff, axis=0)          # [N, E], own rows first
        xT = np.ascontiguousarray(xb.T)            # [E, N]
        gaT = np.roll(eg_bf[b][qoff:qoff + NQ, :], -qoff, axis=1).T  # [N(k), NQ]
        gbT = np.roll(gb_bf[b][qoff:qoff + NQ, :], -qoff, axis=1).T
        lgT = np.roll(lg_bf[b][qoff:qoff + NQ, :], -qoff, axis=1).T
        g2T = np.roll(g2_bf[b][qoff:qoff + NQ, :], -qoff, axis=1).T
        gabT = np.concatenate([gaT, gbT], axis=1)
        for kc in range(NKC):
            if kc % 4 == 0:   # must match is_preload()
                sl = slice(128 * kc, 128 * (kc + 1))
                gabT[sl, :NQ] = lgT[sl]
                gabT[sl, NQ:] = g2T[sl]
        gabT = np.ascontiguousarray(gabT)
        m = {"xT": xT, "gabT": gabT, "vecs": vecs_np,
             "ident": np.eye(128, dtype=np.float32)}
        m.update(ws)
        in_maps.append(m)
    return in_maps


_NC_CACHE = []


def kernel(**inputs):
    if not _NC_CACHE:
        _NC_CACHE.append(build_nc())
    nc = _NC_CACHE[0]
    in_maps = host_shard(inputs)
    res = run_bass_kernel_spmd(nc, in_maps, core_ids=list(range(8)))
    out = np.empty((B, N, E), np.float32)
    for core in range(8):
        b, qh = core // 2, core % 2
        out[b, qh * NQ:(qh + 1) * NQ, :] = np.asarray(
            res.results[core]["outT"], np.float32).T
    return out
